# revision 1
# baseline (speedup 1.0000x reference)
"""DRASI encoder (MLP -> GraphConv x2 -> mu/logvar heads) on 8 Trainium2 cores.

Sharding: nodes are split into 8 contiguous shards of 6250. Each core runs the
node-local MLP on its shard (transposed layout, weights as matmul lhsT), the
shards are AllGathered into a full [50000, 128] feature table in DRAM, and
each core processes the edges whose destination lies in its shard:

  - edges are sorted by dst and bucketed into 64-node "groups"; each
    (group, src-half) bucket is padded to whole 128-edge blocks, with the
    block count unified across cores (max) so all 8 cores share one program;
  - dma_gather fetches source rows from the table (int16 indices, so the
    table is addressed as two 25000-row halves);
  - a DVE-built selection matrix S_w[e, s] = w_e * (seg_e == s) turns the
    segment sum into per-block PE matmuls accumulating aggT = msg.T @ S_w
    in PSUM (features x group-nodes), evicted per bucket into an SBUF table;
  - the GraphConv linear layers + relu run on the transposed activations,
    which are PE-transposed back to natural layout only to publish the next
    AllGather table.

Outputs (mu, logvar) are computed per shard and concatenated on the host.
"""
import sys
sys.path.insert(0, '/opt/trn_rl_repo')

import numpy as np
import concourse.bass as bass
import concourse.bacc as bacc
import concourse.mybir as mybir
from concourse.tile import TileContext
from concourse.masks import make_identity
from concourse import bass_utils

P = 128
N_CORES = 8
N_NODES = 50000
IN_DIM = 512
HID = 128
LAT = 32
SHARD = N_NODES // N_CORES          # 6250
HALF = N_NODES // 2                 # 25000
W = 64                              # nodes per segment group (PSUM tile width)
MAXBLK = 48                         # max 128-edge blocks per gather chunk
HCAP = 28                           # max blocks per src-half within a chunk
N_GROUPS = (SHARD + W - 1) // W     # 98
N_TILES = [512] * (SHARD // 512) + ([SHARD % 512] if SHARD % 512 else [])
F32 = mybir.dt.float32
BF16 = mybir.dt.bfloat16
I16 = mybir.dt.int16
import ml_dtypes
NP_BF16 = ml_dtypes.bfloat16


# ---------------------------------------------------------------- host prep --

def _unified_structure(per_core_edges):
    """per_core_edges: list of (src, dst_local, w) sorted by dst_local.
    Returns (chunk_meta, per-core device arrays eidx/eseg/ew)."""
    # bucket edges per core into (group, half)
    buckets = [[[None, None] for _ in range(N_GROUPS)] for _ in range(N_CORES)]
    for c, (src, dstl, wgt) in enumerate(per_core_edges):
        grp = dstl // W
        for g in range(N_GROUPS):
            sel = grp == g
            gs, gd, gw = src[sel], dstl[sel], wgt[sel]
            hi = gs >= HALF
            for h in (0, 1):
                m = hi == bool(h)
                buckets[c][g][h] = (gs[m] - h * HALF, gd[m] - g * W, gw[m])

    # unified block count per (group, half): max over cores, >= 1 block per
    # group total so every group gets an eviction
    B = np.zeros((N_GROUPS, 2), np.int64)
    for g in range(N_GROUPS):
        for h in (0, 1):
            B[g, h] = max((buckets[c][g][h][0].shape[0] + P - 1) // P
                          for c in range(N_CORES))
        if B[g, 0] == 0 and B[g, 1] == 0:
            B[g, 0] = 1

    # pack consecutive groups into chunks of <= MAXBLK blocks, with each
    # src-half capped at HCAP (separate msgL/msgH tiles)
    chunks = []
    cur, cur_lo, cur_hi = [], 0, 0
    for g in range(N_GROUPS):
        lo, hi = int(B[g, 0]), int(B[g, 1])
        if cur and (cur_lo + lo > HCAP or cur_hi + hi > HCAP
                    or cur_lo + cur_hi + lo + hi > MAXBLK):
            chunks.append(cur)
            cur, cur_lo, cur_hi = [], 0, 0
        cur.append(g)
        cur_lo += lo
        cur_hi += hi
    if cur:
        chunks.append(cur)

    chunk_meta = []
    core_idx = [[] for _ in range(N_CORES)]
    core_seg = [[] for _ in range(N_CORES)]
    core_w = [[] for _ in range(N_CORES)]
    for groups in chunks:
        nblk_lo = int(sum(B[g, 0] for g in groups))
        nblk_hi = int(sum(B[g, 1] for g in groups))
        nblk = nblk_lo + nblk_hi
        runs = []
        b = 0
        for h in (0, 1):
            for g in groups:
                nb = int(B[g, h])
                if nb:
                    runs.append((g, h, b, b + nb))
                    b += nb
        chunk_meta.append(dict(nblk=nblk, nblk_lo=nblk_lo, runs=runs,
                               groups=list(groups)))

        for c in range(N_CORES):
            idx_flat = np.zeros(nblk * P, np.int16)
            seg_flat = np.zeros(nblk * P, np.float32)
            w_flat = np.zeros(nblk * P, np.float32)
            for (g, h, b0, b1_) in runs:
                ids, segs, ws = buckets[c][g][h]
                n = ids.shape[0]
                o = b0 * P
                idx_flat[o:o + n] = ids.astype(np.int16)
                seg_flat[o:o + n] = segs.astype(np.float32)
                w_flat[o:o + n] = ws
            idx_t = np.tile(idx_flat.reshape(nblk * 8, 16).T, (8, 1))
            seg_t = seg_flat.reshape(nblk, P).T.astype(NP_BF16).view(np.int16)
            w_t = w_flat.reshape(nblk, P).T.astype(NP_BF16).view(np.int16)
            core_idx[c].append(np.concatenate([idx_t, seg_t, w_t], axis=1))

    edata = [np.ascontiguousarray(np.concatenate(core_idx[c], axis=1))
             for c in range(N_CORES)]
    return chunk_meta, edata


# ------------------------------------------------------------- device build --

def _build(metas, idx_cols, blk_cols):
    nc = bacc.Bacc(None, target_bir_lowering=False, num_devices=N_CORES,
                   num_swdge_queues=2)

    xT = nc.dram_tensor("xT", [IN_DIM, SHARD], BF16, kind="ExternalInput")
    w1T = nc.dram_tensor("w1T", [IN_DIM, HID], BF16, kind="ExternalInput")
    b1 = nc.dram_tensor("b1", [HID, 1], F32, kind="ExternalInput")
    w2T = nc.dram_tensor("w2T", [HID, HID], BF16, kind="ExternalInput")
    b2 = nc.dram_tensor("b2", [HID, 1], F32, kind="ExternalInput")
    conv_wT = nc.dram_tensor("conv_wT", [2, 2, HID, HID], F32, kind="ExternalInput")
    conv_b = nc.dram_tensor("conv_b", [2, HID, 1], F32, kind="ExternalInput")
    headWT = nc.dram_tensor("headWT", [HID, 2 * LAT], F32, kind="ExternalInput")
    head_b = nc.dram_tensor("head_b", [2 * LAT, 1], F32, kind="ExternalInput")
    iota = nc.dram_tensor("iota", [P, W, MAXBLK], BF16, kind="ExternalInput")
    edata = nc.dram_tensor("edata", [P, idx_cols + 2 * blk_cols], I16,
                           kind="ExternalInput")
    muv_out = nc.dram_tensor("muvT", [2 * LAT, SHARD], F32, kind="ExternalOutput")

    ag_in = [nc.dram_tensor(f"ag_in{i}", [SHARD, HID], BF16) for i in range(2)]
    tables = [nc.dram_tensor(f"h_full{i}", [N_NODES, HID], BF16,
                             addr_space="Shared") for i in range(2)]

    with TileContext(nc) as tc:
        with (
            tc.tile_pool(name="const", bufs=1) as cp,
            tc.tile_pool(name="big", bufs=1) as bigp,
            tc.tile_pool(name="work", bufs=3) as wp,
            tc.tile_pool(name="ps_lin", bufs=3, space="PSUM") as ps_lin,
            tc.tile_pool(name="ps_tr", bufs=2, space="PSUM") as ps_tr,
        ):
            # ---- constants ----
            w1t_sb = [cp.tile([P, HID], BF16, tag=f"w1_{k}", name=f"w1t_{k}") for k in range(4)]
            for k in range(4):
                nc.sync.dma_start(out=w1t_sb[k][:], in_=w1T[k * P:(k + 1) * P, :])
            w2t_sb = cp.tile([P, HID], BF16, tag="w2")
            nc.sync.dma_start(out=w2t_sb[:], in_=w2T[:, :])
            cw_sb = [[cp.tile([P, HID], F32, tag=f"cw{l}{m}", name=f"cw_{l}_{m}") for m in range(2)]
                     for l in range(2)]
            for l in range(2):
                for m in range(2):
                    nc.sync.dma_start(out=cw_sb[l][m][:], in_=conv_wT[l, m, :, :])
            b1_sb = cp.tile([P, 1], F32, tag="b1")
            nc.sync.dma_start(out=b1_sb[:], in_=b1[:, :])
            b2_sb = cp.tile([P, 1], F32, tag="b2")
            nc.sync.dma_start(out=b2_sb[:], in_=b2[:, :])
            cb_sb = [cp.tile([P, 1], F32, tag=f"cb{l}", name=f"cb_{l}") for l in range(2)]
            for l in range(2):
                nc.sync.dma_start(out=cb_sb[l][:], in_=conv_b[l, :, :])
            hw_sb = cp.tile([P, 2 * LAT], F32, tag="hw")
            nc.sync.dma_start(out=hw_sb[:], in_=headWT[:, :])
            hb_sb = cp.tile([2 * LAT, 1], F32, tag="hb")
            nc.sync.dma_start(out=hb_sb[:], in_=head_b[:, :])
            iota_sb = cp.tile([P, W, MAXBLK], BF16, tag="iota")
            nc.sync.dma_start(out=iota_sb[:], in_=iota[:, :, :])
            ident = cp.tile([P, P], F32, tag="ident")
            make_identity(nc, ident[:])

            hA = bigp.tile([P, SHARD], F32, tag="hA")   # h2T, then h4T
            hB = bigp.tile([P, SHARD], F32, tag="hB")   # h3T
            aggT = bigp.tile([P, SHARD], F32, tag="aggT")

            def emit_publish_tiles(hT_tile, t_idx, n0, n1, evict="act"):
                while n0 < n1:
                    w_ = min(P, n1 - n0)
                    tr_ps = ps_tr.tile([P, P], F32, space="PSUM", tag="tr",
                                       name="trp")
                    nc.tensor.transpose(out=tr_ps[:w_, :],
                                        in_=hT_tile[:, n0:n0 + w_],
                                        identity=ident[:])
                    nat = wp.tile([P, P], BF16, tag="nat", name="nat")
                    if evict == "act":
                        nc.scalar.activation(
                            out=nat[:w_, :], in_=tr_ps[:w_, :],
                            func=mybir.ActivationFunctionType.Copy)
                    else:
                        nc.vector.tensor_copy(out=nat[:w_, :],
                                              in_=tr_ps[:w_, :])
                    nc.sync.dma_start(out=ag_in[t_idx][n0:n0 + w_, :],
                                      in_=nat[:w_, :])
                    n0 += w_

            def emit_allgather(t_idx):
                nc.gpsimd.collective_compute(
                    "AllGather", mybir.AluOpType.bypass,
                    replica_groups=[list(range(N_CORES))],
                    ins=[ag_in[t_idx][:, :]],
                    outs=[tables[t_idx][:, :]],
                )


            # ---- MLP (bf16 matmuls, f32 psum) ----
            xfp_cm = tc.tile_pool(name="xf", bufs=1)
            xfp = xfp_cm.__enter__()
            xfull = [xfp.tile([P, SHARD], BF16, tag=f"xf{k}", name=f"xf_{k}")
                     for k in range(4)]
            for k in range(4):
                eng = nc.sync if k % 2 == 0 else nc.scalar
                eng.dma_start(out=xfull[k][:],
                              in_=xT[k * P:(k + 1) * P, :])
            col = 0
            for nt in N_TILES:
                h1_ps = ps_lin.tile([P, 512], F32, space="PSUM", tag="lin")
                for k in range(4):
                    nc.tensor.matmul(out=h1_ps[:, :nt], lhsT=w1t_sb[k][:],
                                     rhs=xfull[k][:, col:col + nt],
                                     start=(k == 0), stop=(k == 3))
                h1_sb = wp.tile([P, 512], BF16, tag="h1")
                nc.scalar.activation(out=h1_sb[:, :nt], in_=h1_ps[:, :nt],
                                     func=mybir.ActivationFunctionType.Relu,
                                     bias=b1_sb[:])
                h2_ps = ps_lin.tile([P, 512], F32, space="PSUM", tag="lin")
                nc.tensor.matmul(out=h2_ps[:, :nt], lhsT=w2t_sb[:],
                                 rhs=h1_sb[:, :nt], start=True, stop=True)
                nc.scalar.activation(out=hA[:, col:col + nt], in_=h2_ps[:, :nt],
                                     func=mybir.ActivationFunctionType.Relu,
                                     bias=b2_sb[:])
                emit_publish_tiles(hA, 0, col, col + nt, evict="dve")
                col += nt

            def conv_layer(layer, hT_in, hT_out, table, pub_idx=None,
                           tile_tail=None):
                icol = 0
                for meta in metas:
                    nblk, nblk_lo = meta["nblk"], meta["nblk_lo"]
                    ed_t = wp.tile([P, MAXBLK * 10], I16, tag="ed")
                    nc.sync.dma_start(out=ed_t[:, :nblk * 10],
                                      in_=edata[:, icol:icol + nblk * 10])
                    idx_t = ed_t[:, :nblk * 8]
                    seg_t = ed_t[:, nblk * 8:nblk * 9].bitcast(BF16)
                    w_t = ed_t[:, nblk * 9:nblk * 10].bitcast(BF16)

                    msgL = msgp.tile([P, HCAP, HID], BF16, tag="msgL")
                    msgH = msgp.tile([P, HCAP, HID], BF16, tag="msgH")
                    if nblk_lo:
                        nc.gpsimd.dma_gather(
                            out_ap=msgL[:, :nblk_lo, :], in_ap=table[:HALF, :],
                            idxs_ap=idx_t[:, :nblk_lo * 8],
                            num_idxs=nblk_lo * P, num_idxs_reg=nblk_lo * P,
                            elem_size=HID, single_packet=False,
                            queue_num=0)
                    if nblk - nblk_lo:
                        nh = nblk - nblk_lo
                        nc.gpsimd.dma_gather(
                            out_ap=msgH[:, :nh, :], in_ap=table[HALF:, :],
                            idxs_ap=idx_t[:, nblk_lo * 8:nblk * 8],
                            num_idxs=nh * P, num_idxs_reg=nh * P,
                            elem_size=HID, single_packet=False,
                            queue_num=1)

                    # S_w in [p, s, block] layout: all operands' last dims are
                    # packed, which enables the DVE 2x perf mode
                    s_w = msgp.tile([P, W, MAXBLK], BF16, tag="sw")
                    nc.vector.tensor_tensor(
                        out=s_w[:, :, :nblk],
                        in0=seg_t.unsqueeze(1).to_broadcast([P, W, nblk]),
                        in1=iota_sb[:, :, :nblk],
                        op=mybir.AluOpType.is_equal)
                    nc.vector.tensor_tensor(
                        out=s_w[:, :, :nblk], in0=s_w[:, :, :nblk],
                        in1=w_t.unsqueeze(1).to_broadcast([P, W, nblk]),
                        op=mybir.AluOpType.mult)

                    # one psum + one eviction per group: a group's lo and hi
                    # runs accumulate into the same tile
                    by_group = {}
                    for (g, h, b0, b1_) in meta["runs"]:
                        by_group.setdefault(g, []).append((h, b0, b1_))
                    for g in meta["groups"]:
                        ps = ps_agg.tile([P, W], F32, space="PSUM", tag="agg")
                        blocks = [(h, b) for (h, b0, b1_) in by_group[g]
                                  for b in range(b0, b1_)]
                        for i, (h, b) in enumerate(blocks):
                            mt = msgL[:, b, :] if h == 0 else \
                                 msgH[:, b - nblk_lo, :]
                            nc.tensor.matmul(out=ps[:], lhsT=mt,
                                             rhs=s_w[:, :, b],
                                             start=(i == 0),
                                             stop=(i == len(blocks) - 1))
                        gw = min(W, SHARD - g * W)
                        nc.scalar.activation(
                            out=aggT[:, g * W:g * W + gw], in_=ps[:, :gw],
                            func=mybir.ActivationFunctionType.Copy)
                    icol += nblk * 10

                col = 0
                for nt in N_TILES:
                    ps = ps_lin.tile([P, 512], F32, space="PSUM", tag="lin")
                    nc.tensor.matmul(out=ps[:, :nt], lhsT=cw_sb[layer][0][:],
                                     rhs=aggT[:, col:col + nt],
                                     start=True, stop=False)
                    nc.tensor.matmul(out=ps[:, :nt], lhsT=cw_sb[layer][1][:],
                                     rhs=hT_in[:, col:col + nt],
                                     start=False, stop=True)
                    nc.scalar.activation(out=hT_out[:, col:col + nt],
                                         in_=ps[:, :nt],
                                         func=mybir.ActivationFunctionType.Relu,
                                         bias=cb_sb[layer][:])
                    if pub_idx is not None:
                        emit_publish_tiles(hT_out, pub_idx, col, col + nt)
                    if tile_tail is not None:
                        tile_tail(col, nt)
                    col += nt

            xfp_cm.__exit__(None, None, None)
            msgp_cm = tc.tile_pool(name="msgp", bufs=2)
            msgp = msgp_cm.__enter__()
            ps_agg_cm = tc.tile_pool(name="ps_agg", bufs=3, space="PSUM")
            ps_agg = ps_agg_cm.__enter__()
            emit_allgather(0)
            conv_layer(0, hA, hB, tables[0], pub_idx=1)
            # ---- heads fused into conv2's linear phase ----
            muvT = bigp.tile([2 * LAT, SHARD], F32, tag="muvT")

            def head_tail(col, nt):
                ps = ps_lin.tile([2 * LAT, 512], F32, space="PSUM", tag="lin",
                                 name="headps")
                nc.tensor.matmul(out=ps[:, :nt], lhsT=hw_sb[:],
                                 rhs=hA[:, col:col + nt], start=True, stop=True)
                nc.vector.tensor_tensor(
                    out=muvT[:, col:col + nt], in0=ps[:, :nt],
                    in1=hb_sb[:].to_broadcast([2 * LAT, nt]),
                    op=mybir.AluOpType.add)

            emit_allgather(1)
            conv_layer(1, hB, hA, tables[1], tile_tail=head_tail)

            msgp_cm.__exit__(None, None, None)
            ps_agg_cm.__exit__(None, None, None)
            nc.sync.dma_start(out=muv_out[:, :], in_=muvT[:])

    nc.finalize()
    return nc


# -------------------------------------------------------------------- driver --

_CACHE = {}


def _get_compiled(x, edge_index, edge_attr, weights):
    src = np.asarray(edge_index[0]).astype(np.int64)
    dst = np.asarray(edge_index[1]).astype(np.int64)
    wgt = np.asarray(edge_attr, dtype=np.float32)
    x = np.asarray(x, dtype=np.float32)

    per_core_edges = []
    for c in range(N_CORES):
        sel = (dst >= c * SHARD) & (dst < (c + 1) * SHARD)
        s, d, wv = src[sel], dst[sel] - c * SHARD, wgt[sel]
        order = np.argsort(d, kind="stable")
        per_core_edges.append((s[order], d[order], wv[order]))

    metas, edata = _unified_structure(per_core_edges)
    idx_cols = sum(m["nblk"] * 8 for m in metas)
    blk_cols = sum(m["nblk"] for m in metas)

    nc = _build(metas, idx_cols, blk_cols)

    (W1, b1, W2, b2, g1_rel_W, g1_rel_b, g1_root_W,
     g2_rel_W, g2_rel_b, g2_root_W, mu_W, mu_b, lv_W, lv_b) = [
        np.asarray(w, dtype=np.float32) for w in weights]

    conv_wT = np.stack([
        np.stack([g1_rel_W.T, g1_root_W.T]),
        np.stack([g2_rel_W.T, g2_root_W.T]),
    ]).copy()
    conv_b = np.stack([g1_rel_b[:, None], g2_rel_b[:, None]]).copy()
    headWT = np.ascontiguousarray(np.concatenate([mu_W, lv_W], axis=0).T)
    head_b = np.concatenate([mu_b, lv_b])[:, None].copy()
    iota = np.ascontiguousarray(np.broadcast_to(
        np.arange(W, dtype=np.float32)[None, :, None],
        (P, W, MAXBLK)).astype(NP_BF16))

    common = dict(
        w1T=np.ascontiguousarray(W1.T.astype(NP_BF16)), b1=b1[:, None].copy(),
        w2T=np.ascontiguousarray(W2.T.astype(NP_BF16)), b2=b2[:, None].copy(),
        conv_wT=conv_wT, conv_b=conv_b, headWT=headWT, head_b=head_b,
        iota=iota,
    )
    in_maps = []
    for c in range(N_CORES):
        m = dict(common)
        m["xT"] = np.ascontiguousarray(x[c * SHARD:(c + 1) * SHARD, :].T.astype(NP_BF16))
        m["edata"] = edata[c]
        in_maps.append(m)
    return nc, in_maps


def kernel(x, edge_index, edge_attr,
           W1, b1, W2, b2,
           g1_rel_W, g1_rel_b, g1_root_W,
           g2_rel_W, g2_rel_b, g2_root_W,
           mu_W, mu_b, lv_W, lv_b):
    weights = (W1, b1, W2, b2, g1_rel_W, g1_rel_b, g1_root_W,
               g2_rel_W, g2_rel_b, g2_root_W, mu_W, mu_b, lv_W, lv_b)
    nc, in_maps = _get_compiled(x, edge_index, edge_attr, weights)
    res = bass_utils.run_bass_kernel_spmd(nc, in_maps,
                                          core_ids=list(range(N_CORES)))
    muvT = np.concatenate([res.results[c]["muvT"] for c in range(N_CORES)],
                          axis=1)
    return (np.ascontiguousarray(muvT[:LAT, :].T),
            np.ascontiguousarray(muvT[LAT:, :].T))



# revision 7
# speedup vs baseline: 1.1237x; 1.1237x over previous
"""DRASI encoder (MLP -> GraphConv x2 -> mu/logvar heads) on 8 Trainium2 cores.

Sharding: nodes are split into 8 contiguous shards of 6250. Each core runs the
node-local MLP on its shard (transposed layout, weights as matmul lhsT), the
shards are AllGathered into a full [50000, 128] feature table in DRAM, and
each core processes the edges whose destination lies in its shard:

  - edges are sorted by dst and bucketed into 64-node "groups"; each
    (group, src-half) bucket is padded to whole 128-edge blocks, with the
    block count unified across cores (max) so all 8 cores share one program;
  - dma_gather fetches source rows from the table (int16 indices, so the
    table is addressed as two 25000-row halves);
  - a DVE-built selection matrix S_w[e, s] = w_e * (seg_e == s) turns the
    segment sum into per-block PE matmuls accumulating aggT = msg.T @ S_w
    in PSUM (features x group-nodes), evicted per bucket into an SBUF table;
  - the GraphConv linear layers + relu run on the transposed activations
    (all bf16 operands so the PE runs at full rate), interleaved per
    512-column tile with the remaining gathers; publishes go through a
    natural-layout staging tile and batched DMAs.

Outputs (mu, logvar) are computed per shard and concatenated on the host.
"""
import sys
sys.path.insert(0, '/opt/trn_rl_repo')

import numpy as np
import concourse.bass as bass
import concourse.bacc as bacc
import concourse.mybir as mybir
from concourse.tile import TileContext
from concourse.masks import make_identity
from concourse import bass_utils

P = 128
N_CORES = 8
N_NODES = 50000
IN_DIM = 512
HID = 128
LAT = 32
SHARD = N_NODES // N_CORES          # 6250
HALF = N_NODES // 2                 # 25000
W = 64                              # nodes per segment group (PSUM tile width)
MAXBLK = 48                         # max 128-edge blocks per gather chunk
HCAP = 28                           # max blocks per src-half within a chunk
N_GROUPS = (SHARD + W - 1) // W     # 98
N_TILES = [512] * (SHARD // 512) + ([SHARD % 512] if SHARD % 512 else [])
NB_STAGE = (SHARD + P - 1) // P     # 49 row-blocks in the staging tile
F32 = mybir.dt.float32
BF16 = mybir.dt.bfloat16
I16 = mybir.dt.int16
import ml_dtypes
NP_BF16 = ml_dtypes.bfloat16


# ---------------------------------------------------------------- host prep --

def _unified_structure(per_core_edges):
    """per_core_edges: list of (src, dst_local, w) sorted by dst_local.
    Returns (chunk_meta, per-core device arrays eidx/eseg/ew)."""
    # bucket edges per core into (group, half)
    buckets = [[[None, None] for _ in range(N_GROUPS)] for _ in range(N_CORES)]
    for c, (src, dstl, wgt) in enumerate(per_core_edges):
        grp = dstl // W
        for g in range(N_GROUPS):
            sel = grp == g
            gs, gd, gw = src[sel], dstl[sel], wgt[sel]
            hi = gs >= HALF
            for h in (0, 1):
                m = hi == bool(h)
                buckets[c][g][h] = (gs[m] - h * HALF, gd[m] - g * W, gw[m])

    # unified block count per (group, half): max over cores, >= 1 block per
    # group total so every group gets an eviction
    B = np.zeros((N_GROUPS, 2), np.int64)
    for g in range(N_GROUPS):
        for h in (0, 1):
            B[g, h] = max((buckets[c][g][h][0].shape[0] + P - 1) // P
                          for c in range(N_CORES))
        if B[g, 0] == 0 and B[g, 1] == 0:
            B[g, 0] = 1

    # pack consecutive groups into chunks of <= MAXBLK blocks, with each
    # src-half capped at HCAP (separate msgL/msgH tiles)
    chunks = []
    cur, cur_lo, cur_hi = [], 0, 0
    for g in range(N_GROUPS):
        lo, hi = int(B[g, 0]), int(B[g, 1])
        if cur and (cur_lo + lo > HCAP or cur_hi + hi > HCAP
                    or cur_lo + cur_hi + lo + hi > MAXBLK):
            chunks.append(cur)
            cur, cur_lo, cur_hi = [], 0, 0
        cur.append(g)
        cur_lo += lo
        cur_hi += hi
    if cur:
        chunks.append(cur)

    chunk_meta = []
    core_idx = [[] for _ in range(N_CORES)]
    for groups in chunks:
        nblk_lo = int(sum(B[g, 0] for g in groups))
        nblk_hi = int(sum(B[g, 1] for g in groups))
        nblk = nblk_lo + nblk_hi
        runs = []
        b = 0
        for h in (0, 1):
            for g in groups:
                nb = int(B[g, h])
                if nb:
                    runs.append((g, h, b, b + nb))
                    b += nb
        chunk_meta.append(dict(nblk=nblk, nblk_lo=nblk_lo, runs=runs,
                               groups=list(groups)))

        for c in range(N_CORES):
            idx_flat = np.zeros(nblk * P, np.int16)
            seg_flat = np.zeros(nblk * P, np.float32)
            w_flat = np.zeros(nblk * P, np.float32)
            for (g, h, b0, b1_) in runs:
                ids, segs, ws = buckets[c][g][h]
                n = ids.shape[0]
                o = b0 * P
                idx_flat[o:o + n] = ids.astype(np.int16)
                seg_flat[o:o + n] = segs.astype(np.float32)
                w_flat[o:o + n] = ws
            idx_t = np.tile(idx_flat.reshape(nblk * 8, 16).T, (8, 1))
            seg_t = seg_flat.reshape(nblk, P).T.astype(NP_BF16).view(np.int16)
            w_t = w_flat.reshape(nblk, P).T.astype(NP_BF16).view(np.int16)
            core_idx[c].append(np.concatenate([idx_t, seg_t, w_t], axis=1))

    edata = [np.ascontiguousarray(np.concatenate(core_idx[c], axis=1))
             for c in range(N_CORES)]
    return chunk_meta, edata


# ------------------------------------------------------------- device build --

def _build(metas, ecols):
    nc = bacc.Bacc(None, target_bir_lowering=False, num_devices=N_CORES,
                   num_swdge_queues=2)

    xT = nc.dram_tensor("xT", [IN_DIM, SHARD], BF16, kind="ExternalInput")
    w1T = nc.dram_tensor("w1T", [IN_DIM, HID], BF16, kind="ExternalInput")
    b1 = nc.dram_tensor("b1", [HID, 1], F32, kind="ExternalInput")
    w2T = nc.dram_tensor("w2T", [HID, HID], BF16, kind="ExternalInput")
    b2 = nc.dram_tensor("b2", [HID, 1], F32, kind="ExternalInput")
    conv_wT = nc.dram_tensor("conv_wT", [2, 2, HID, HID], BF16, kind="ExternalInput")
    conv_b = nc.dram_tensor("conv_b", [2, HID, 1], F32, kind="ExternalInput")
    headWT = nc.dram_tensor("headWT", [HID, 2 * LAT], BF16, kind="ExternalInput")
    head_b = nc.dram_tensor("head_b", [2 * LAT, 1], F32, kind="ExternalInput")
    iota = nc.dram_tensor("iota", [P, W, MAXBLK], BF16, kind="ExternalInput")
    edata = nc.dram_tensor("edata", [P, ecols], I16, kind="ExternalInput")
    muv_out = nc.dram_tensor("muvT", [2 * LAT, SHARD], F32, kind="ExternalOutput")

    ag_in = [nc.dram_tensor(f"ag_in{i}", [SHARD, HID], BF16) for i in range(2)]
    tables = [nc.dram_tensor(f"h_full{i}", [N_NODES, HID], BF16,
                             addr_space="Shared") for i in range(2)]

    chunk_ioff = []
    io = 0
    for m in metas:
        chunk_ioff.append(io)
        io += m["nblk"] * 10
    assert io == ecols

    with TileContext(nc) as tc:
        with (
            tc.tile_pool(name="const", bufs=1) as cp,
            tc.tile_pool(name="big", bufs=1) as bigp,
            tc.tile_pool(name="work", bufs=3) as wp,
            tc.tile_pool(name="msgp", bufs=2) as msgp,
            tc.tile_pool(name="swp", bufs=3) as swp,
            tc.tile_pool(name="ps_lin", bufs=3, space="PSUM") as ps_lin,
            tc.tile_pool(name="ps_tr", bufs=2, space="PSUM") as ps_tr,
            tc.tile_pool(name="ps_agg", bufs=3, space="PSUM") as ps_agg,
        ):
            # ---- constants ----
            w1t_sb = [cp.tile([P, HID], BF16, tag=f"w1_{k}", name=f"w1t_{k}") for k in range(4)]
            for k in range(4):
                nc.sync.dma_start(out=w1t_sb[k][:], in_=w1T[k * P:(k + 1) * P, :])
            w2t_sb = cp.tile([P, HID], BF16, tag="w2")
            nc.sync.dma_start(out=w2t_sb[:], in_=w2T[:, :])
            cw_sb = [[cp.tile([P, HID], BF16, tag=f"cw{l}{m}", name=f"cw_{l}_{m}") for m in range(2)]
                     for l in range(2)]
            for l in range(2):
                for m in range(2):
                    nc.scalar.dma_start(out=cw_sb[l][m][:], in_=conv_wT[l, m, :, :])
            b1_sb = cp.tile([P, 1], F32, tag="b1")
            nc.scalar.dma_start(out=b1_sb[:], in_=b1[:, :])
            b2_sb = cp.tile([P, 1], F32, tag="b2")
            nc.scalar.dma_start(out=b2_sb[:], in_=b2[:, :])
            cb_sb = [cp.tile([P, 1], F32, tag=f"cb{l}", name=f"cb_{l}") for l in range(2)]
            for l in range(2):
                nc.scalar.dma_start(out=cb_sb[l][:], in_=conv_b[l, :, :])
            hw_sb = cp.tile([P, 2 * LAT], BF16, tag="hw")
            nc.scalar.dma_start(out=hw_sb[:], in_=headWT[:, :])
            hb_sb = cp.tile([2 * LAT, 1], F32, tag="hb")
            nc.scalar.dma_start(out=hb_sb[:], in_=head_b[:, :])
            iota_sb = cp.tile([P, W, MAXBLK], BF16, tag="iota")
            nc.sync.dma_start(out=iota_sb[:], in_=iota[:, :, :])
            ident = cp.tile([P, P], F32, tag="ident")
            make_identity(nc, ident[:])
            identb = cp.tile([P, P], BF16, tag="identb")
            nc.vector.tensor_copy(out=identb[:], in_=ident[:])

            # all edge metadata (shared by both conv layers) in one load
            edall = bigp.tile([P, ecols], I16, tag="edall")
            nc.scalar.dma_start(out=edall[:], in_=edata[:, :])

            hA = bigp.tile([P, SHARD], BF16, tag="hA")   # h2T, then h4T
            hB = bigp.tile([P, SHARD], BF16, tag="hB")   # h3T
            aggT = bigp.tile([P, SHARD], BF16, tag="aggT")
            stage = bigp.tile([P, NB_STAGE * P], BF16, tag="stage")

            def emit_publish(hT_tile, t_idx, col, nt):
                # transpose 128-col blocks into natural layout in `stage`,
                # then one batched DMA per tile into ag_in
                b0 = col // P
                nfull = nt // P
                nblks = (nt + P - 1) // P
                for i in range(nblks):
                    w_ = min(P, nt - i * P)
                    tr_ps = ps_tr.tile([P, P], BF16, space="PSUM", tag="tr",
                                       name="trp")
                    nc.tensor.transpose(out=tr_ps[:w_, :],
                                        in_=hT_tile[:, col + i * P:col + i * P + w_],
                                        identity=identb[:])
                    nc.vector.tensor_copy(
                        out=stage[:w_, (b0 + i) * P:(b0 + i + 1) * P],
                        in_=tr_ps[:w_, :])
                if nfull:
                    nc.sync.dma_start(
                        out=ag_in[t_idx][col:col + nfull * P, :].rearrange(
                            "(b p) f -> p b f", p=P),
                        in_=stage[:, col:col + nfull * P].rearrange(
                            "p (b f) -> p b f", f=P))
                if nt % P:
                    r0 = col + nfull * P
                    rb = b0 + nfull
                    nc.sync.dma_start(
                        out=ag_in[t_idx][r0:r0 + nt % P, :],
                        in_=stage[:nt % P, rb * P:(rb + 1) * P])

            def emit_allgather(t_idx):
                nc.gpsimd.collective_compute(
                    "AllGather", mybir.AluOpType.bypass,
                    replica_groups=[list(range(N_CORES))],
                    ins=[ag_in[t_idx][:, :]],
                    outs=[tables[t_idx][:, :]],
                )

            def build_sw(ci):
                meta = metas[ci]
                nblk = meta["nblk"]
                io = chunk_ioff[ci]
                seg_t = edall[:, io + nblk * 8:io + nblk * 9].bitcast(BF16)
                w_t = edall[:, io + nblk * 9:io + nblk * 10].bitcast(BF16)
                s_w = swp.tile([P, W, MAXBLK], BF16, tag="sw")
                nc.vector.tensor_tensor(
                    out=s_w[:, :, :nblk],
                    in0=seg_t.unsqueeze(1).to_broadcast([P, W, nblk]),
                    in1=iota_sb[:, :, :nblk],
                    op=mybir.AluOpType.is_equal)
                nc.vector.tensor_tensor(
                    out=s_w[:, :, :nblk], in0=s_w[:, :, :nblk],
                    in1=w_t.unsqueeze(1).to_broadcast([P, W, nblk]),
                    op=mybir.AluOpType.mult)
                return s_w

            # ---- MLP (bf16 matmuls, f32 psum) ----
            xfp_cm = tc.tile_pool(name="xf", bufs=1)
            xfp = xfp_cm.__enter__()
            xfull = [xfp.tile([P, SHARD], BF16, tag=f"xf{k}", name=f"xf_{k}")
                     for k in range(4)]
            # column-chunked loads so the first MLP tile starts early
            x_engs = [nc.sync, nc.scalar, nc.sync, nc.scalar]
            CCH = [(0, 1536), (1536, 3072), (3072, 4608), (4608, SHARD)]
            for (c0, c1) in CCH:
                for k in range(4):
                    x_engs[k % 4].dma_start(out=xfull[k][:, c0:c1],
                                            in_=xT[k * P:(k + 1) * P, c0:c1])
            col = 0
            for nt in N_TILES:
                h1_ps = ps_lin.tile([P, 512], F32, space="PSUM", tag="lin")
                for k in range(4):
                    nc.tensor.matmul(out=h1_ps[:, :nt], lhsT=w1t_sb[k][:],
                                     rhs=xfull[k][:, col:col + nt],
                                     start=(k == 0), stop=(k == 3))
                h1_sb = wp.tile([P, 512], BF16, tag="h1")
                nc.scalar.activation(out=h1_sb[:, :nt], in_=h1_ps[:, :nt],
                                     func=mybir.ActivationFunctionType.Relu,
                                     bias=b1_sb[:])
                h2_ps = ps_lin.tile([P, 512], F32, space="PSUM", tag="lin")
                nc.tensor.matmul(out=h2_ps[:, :nt], lhsT=w2t_sb[:],
                                 rhs=h1_sb[:, :nt], start=True, stop=True)
                nc.scalar.activation(out=hA[:, col:col + nt], in_=h2_ps[:, :nt],
                                     func=mybir.ActivationFunctionType.Relu,
                                     bias=b2_sb[:])
                emit_publish(hA, 0, col, nt)
                col += nt

            def conv_layer(layer, hT_in, hT_out, table, pub_idx=None,
                           heads=False, sw_cache=None):
                tiles_done = 0
                groups_done = 0

                def emit_linear(t):
                    col = 512 * t
                    nt = N_TILES[t]
                    ps = ps_lin.tile([P, 512], F32, space="PSUM", tag="lin")
                    nc.tensor.matmul(out=ps[:, :nt], lhsT=cw_sb[layer][0][:],
                                     rhs=aggT[:, col:col + nt],
                                     start=True, stop=False)
                    nc.tensor.matmul(out=ps[:, :nt], lhsT=cw_sb[layer][1][:],
                                     rhs=hT_in[:, col:col + nt],
                                     start=False, stop=True)
                    nc.scalar.activation(out=hT_out[:, col:col + nt],
                                         in_=ps[:, :nt],
                                         func=mybir.ActivationFunctionType.Relu,
                                         bias=cb_sb[layer][:])
                    if pub_idx is not None:
                        emit_publish(hT_out, pub_idx, col, nt)
                    if heads:
                        psh = ps_lin.tile([2 * LAT, 512], F32, space="PSUM",
                                          tag="lin", name="headps")
                        nc.tensor.matmul(out=psh[:, :nt], lhsT=hw_sb[:],
                                         rhs=hT_out[:, col:col + nt],
                                         start=True, stop=True)
                        muv = wp.tile([2 * LAT, 512], F32, tag="muv",
                                      name="muv")
                        nc.vector.tensor_tensor(
                            out=muv[:, :nt], in0=psh[:, :nt],
                            in1=hb_sb[:].to_broadcast([2 * LAT, nt]),
                            op=mybir.AluOpType.add)
                        nc.sync.dma_start(out=muv_out[:, col:col + nt],
                                          in_=muv[:, :nt])

                for ci, meta in enumerate(metas):
                    nblk, nblk_lo = meta["nblk"], meta["nblk_lo"]
                    io = chunk_ioff[ci]
                    idx_t = edall[:, io:io + nblk * 8]

                    msgL = msgp.tile([P, HCAP, HID], BF16, tag="msgL")
                    msgH = msgp.tile([P, HCAP, HID], BF16, tag="msgH")
                    if nblk_lo:
                        nc.gpsimd.dma_gather(
                            out_ap=msgL[:, :nblk_lo, :], in_ap=table[:HALF, :],
                            idxs_ap=idx_t[:, :nblk_lo * 8],
                            num_idxs=nblk_lo * P, num_idxs_reg=nblk_lo * P,
                            elem_size=HID, single_packet=False,
                            queue_num=0)
                    if nblk - nblk_lo:
                        nh = nblk - nblk_lo
                        nc.gpsimd.dma_gather(
                            out_ap=msgH[:, :nh, :], in_ap=table[HALF:, :],
                            idxs_ap=idx_t[:, nblk_lo * 8:nblk * 8],
                            num_idxs=nh * P, num_idxs_reg=nh * P,
                            elem_size=HID, single_packet=False,
                            queue_num=1)

                    if sw_cache is not None and ci in sw_cache:
                        s_w = sw_cache.pop(ci)
                    else:
                        s_w = build_sw(ci)

                    # one psum + one eviction per group: a group's lo and hi
                    # runs accumulate into the same tile
                    by_group = {}
                    for (g, h, b0, b1_) in meta["runs"]:
                        by_group.setdefault(g, []).append((h, b0, b1_))
                    for g in meta["groups"]:
                        ps = ps_agg.tile([P, W], F32, space="PSUM", tag="agg")
                        blocks = [(h, b) for (h, b0, b1_) in by_group[g]
                                  for b in range(b0, b1_)]
                        for i, (h, b) in enumerate(blocks):
                            mt = msgL[:, b, :] if h == 0 else \
                                 msgH[:, b - nblk_lo, :]
                            nc.tensor.matmul(out=ps[:], lhsT=mt,
                                             rhs=s_w[:, :, b],
                                             start=(i == 0),
                                             stop=(i == len(blocks) - 1))
                        gw = min(W, SHARD - g * W)
                        nc.scalar.activation(
                            out=aggT[:, g * W:g * W + gw], in_=ps[:, :gw],
                            func=mybir.ActivationFunctionType.Copy)
                    groups_done += len(meta["groups"])

                    while (tiles_done < len(N_TILES)
                           and groups_done * W >= tiles_done * 512
                           + N_TILES[tiles_done]):
                        emit_linear(tiles_done)
                        tiles_done += 1
                while tiles_done < len(N_TILES):
                    emit_linear(tiles_done)
                    tiles_done += 1

            xfp_cm.__exit__(None, None, None)

            # prebuild the first S_w tiles while the AllGather is in flight
            sw_cache = {ci: build_sw(ci) for ci in range(min(3, len(metas)))}
            emit_allgather(0)
            conv_layer(0, hA, hB, tables[0], pub_idx=1, sw_cache=sw_cache)
            sw_cache = {ci: build_sw(ci) for ci in range(min(3, len(metas)))}
            emit_allgather(1)
            conv_layer(1, hB, hA, tables[1], heads=True, sw_cache=sw_cache)

    nc.finalize()
    return nc


# -------------------------------------------------------------------- driver --

def _get_compiled(x, edge_index, edge_attr, weights):
    src = np.asarray(edge_index[0]).astype(np.int64)
    dst = np.asarray(edge_index[1]).astype(np.int64)
    wgt = np.asarray(edge_attr, dtype=np.float32)
    x = np.asarray(x, dtype=np.float32)

    per_core_edges = []
    for c in range(N_CORES):
        sel = (dst >= c * SHARD) & (dst < (c + 1) * SHARD)
        s, d, wv = src[sel], dst[sel] - c * SHARD, wgt[sel]
        order = np.argsort(d, kind="stable")
        per_core_edges.append((s[order], d[order], wv[order]))

    metas, edata = _unified_structure(per_core_edges)
    ecols = sum(m["nblk"] * 10 for m in metas)

    nc = _build(metas, ecols)

    (W1, b1, W2, b2, g1_rel_W, g1_rel_b, g1_root_W,
     g2_rel_W, g2_rel_b, g2_root_W, mu_W, mu_b, lv_W, lv_b) = [
        np.asarray(w, dtype=np.float32) for w in weights]

    conv_wT = np.stack([
        np.stack([g1_rel_W.T, g1_root_W.T]),
        np.stack([g2_rel_W.T, g2_root_W.T]),
    ]).astype(NP_BF16).copy()
    conv_b = np.stack([g1_rel_b[:, None], g2_rel_b[:, None]]).copy()
    headWT = np.ascontiguousarray(
        np.concatenate([mu_W, lv_W], axis=0).T.astype(NP_BF16))
    head_b = np.concatenate([mu_b, lv_b])[:, None].copy()
    iota = np.ascontiguousarray(np.broadcast_to(
        np.arange(W, dtype=np.float32)[None, :, None],
        (P, W, MAXBLK)).astype(NP_BF16))

    common = dict(
        w1T=np.ascontiguousarray(W1.T.astype(NP_BF16)), b1=b1[:, None].copy(),
        w2T=np.ascontiguousarray(W2.T.astype(NP_BF16)), b2=b2[:, None].copy(),
        conv_wT=conv_wT, conv_b=conv_b, headWT=headWT, head_b=head_b,
        iota=iota,
    )
    in_maps = []
    for c in range(N_CORES):
        m = dict(common)
        m["xT"] = np.ascontiguousarray(x[c * SHARD:(c + 1) * SHARD, :].T.astype(NP_BF16))
        m["edata"] = edata[c]
        in_maps.append(m)
    return nc, in_maps


def kernel(x, edge_index, edge_attr,
           W1, b1, W2, b2,
           g1_rel_W, g1_rel_b, g1_root_W,
           g2_rel_W, g2_rel_b, g2_root_W,
           mu_W, mu_b, lv_W, lv_b):
    weights = (W1, b1, W2, b2, g1_rel_W, g1_rel_b, g1_root_W,
               g2_rel_W, g2_rel_b, g2_root_W, mu_W, mu_b, lv_W, lv_b)
    nc, in_maps = _get_compiled(x, edge_index, edge_attr, weights)
    res = bass_utils.run_bass_kernel_spmd(nc, in_maps,
                                          core_ids=list(range(N_CORES)))
    muvT = np.concatenate([res.results[c]["muvT"] for c in range(N_CORES)],
                          axis=1)
    return (np.ascontiguousarray(muvT[:LAT, :].T),
            np.ascontiguousarray(muvT[LAT:, :].T))


# revision 18
# speedup vs baseline: 1.1304x; 1.0060x over previous
"""DRASI encoder (MLP -> GraphConv x2 -> mu/logvar heads) on 8 Trainium2 cores.

Sharding: nodes are split into 8 contiguous shards of 6250. Each core runs the
node-local MLP on its shard (transposed layout, weights as matmul lhsT), the
shards are AllGathered into a full [50000, 128] feature table in DRAM, and
each core processes the edges whose destination lies in its shard:

  - edges are sorted by dst and bucketed into 64-node "groups"; each
    (group, src-half) bucket is padded to whole 128-edge blocks, with the
    block count unified across cores (max) so all 8 cores share one program;
  - dma_gather fetches source rows from the table (int16 indices, so the
    table is addressed as two 25000-row halves);
  - a DVE-built selection matrix S_w[e, s] = w_e * (seg_e == s) turns the
    segment sum into per-block PE matmuls accumulating aggT = msg.T @ S_w
    in PSUM (features x group-nodes), evicted per bucket into an SBUF table;
  - the GraphConv linear layers + relu run on the transposed activations
    (all bf16 operands so the PE runs at full rate), interleaved per
    512-column tile with the remaining gathers; publishes go through a
    natural-layout staging tile and batched DMAs.

Outputs (mu, logvar) are computed per shard and concatenated on the host.
"""
import sys
sys.path.insert(0, '/opt/trn_rl_repo')

import numpy as np
import concourse.bass as bass
import concourse.bacc as bacc
import concourse.mybir as mybir
from concourse.tile import TileContext
from concourse.masks import make_identity
from concourse import bass_utils

P = 128
N_CORES = 8
N_NODES = 50000
IN_DIM = 512
HID = 128
LAT = 32
SHARD = N_NODES // N_CORES          # 6250
HALF = N_NODES // 2                 # 25000
W = 64                              # nodes per segment group (PSUM tile width)
MAXBLK = 48                         # max 128-edge blocks per gather chunk
HCAP = 28                           # max blocks per src-half within a chunk
N_GROUPS = (SHARD + W - 1) // W     # 98
N_TILES = [512] * (SHARD // 512) + ([SHARD % 512] if SHARD % 512 else [])
NB_STAGE = (SHARD + P - 1) // P     # 49 row-blocks in the staging tile
F32 = mybir.dt.float32
BF16 = mybir.dt.bfloat16
I16 = mybir.dt.int16
import ml_dtypes
NP_BF16 = ml_dtypes.bfloat16


# ---------------------------------------------------------------- host prep --

def _unified_structure(per_core_edges):
    """per_core_edges: list of (src, dst_local, w) sorted by dst_local.
    Returns (chunk_meta, per-core device arrays eidx/eseg/ew)."""
    # bucket edges per core into (group, half)
    buckets = [[[None, None] for _ in range(N_GROUPS)] for _ in range(N_CORES)]
    for c, (src, dstl, wgt) in enumerate(per_core_edges):
        grp = dstl // W
        for g in range(N_GROUPS):
            sel = grp == g
            gs, gd, gw = src[sel], dstl[sel], wgt[sel]
            hi = gs >= HALF
            for h in (0, 1):
                m = hi == bool(h)
                buckets[c][g][h] = (gs[m] - h * HALF, gd[m] - g * W, gw[m])

    # unified block count per (group, half): max over cores, >= 1 block per
    # group total so every group gets an eviction
    B = np.zeros((N_GROUPS, 2), np.int64)
    for g in range(N_GROUPS):
        for h in (0, 1):
            B[g, h] = max((buckets[c][g][h][0].shape[0] + P - 1) // P
                          for c in range(N_CORES))
        if B[g, 0] == 0 and B[g, 1] == 0:
            B[g, 0] = 1

    # pack consecutive groups into chunks of <= MAXBLK blocks, with each
    # src-half capped at HCAP (separate msgL/msgH tiles); the last group is
    # kept as its own small chunk so the window's serial tail after the final
    # gather is short
    chunks = []
    cur, cur_lo, cur_hi = [], 0, 0
    for g in range(N_GROUPS - 1):
        lo, hi = int(B[g, 0]), int(B[g, 1])
        if cur and (cur_lo + lo > HCAP or cur_hi + hi > HCAP
                    or cur_lo + cur_hi + lo + hi > MAXBLK):
            chunks.append(cur)
            cur, cur_lo, cur_hi = [], 0, 0
        cur.append(g)
        cur_lo += lo
        cur_hi += hi
    if cur:
        chunks.append(cur)
    chunks.append([N_GROUPS - 1])

    chunk_meta = []
    core_idx = [[] for _ in range(N_CORES)]
    for groups in chunks:
        nblk_lo = int(sum(B[g, 0] for g in groups))
        nblk_hi = int(sum(B[g, 1] for g in groups))
        nblk = nblk_lo + nblk_hi
        runs = []
        b = 0
        for h in (0, 1):
            for g in groups:
                nb = int(B[g, h])
                if nb:
                    runs.append((g, h, b, b + nb))
                    b += nb
        chunk_meta.append(dict(nblk=nblk, nblk_lo=nblk_lo, runs=runs,
                               groups=list(groups)))

        for c in range(N_CORES):
            idx_flat = np.zeros(nblk * P, np.int16)
            seg_flat = np.zeros(nblk * P, np.float32)
            w_flat = np.zeros(nblk * P, np.float32)
            for (g, h, b0, b1_) in runs:
                ids, segs, ws = buckets[c][g][h]
                n = ids.shape[0]
                o = b0 * P
                idx_flat[o:o + n] = ids.astype(np.int16)
                seg_flat[o:o + n] = segs.astype(np.float32)
                w_flat[o:o + n] = ws
            idx_t = np.tile(idx_flat.reshape(nblk * 8, 16).T, (8, 1))
            seg_t = seg_flat.reshape(nblk, P).T.astype(NP_BF16).view(np.int16)
            w_t = w_flat.reshape(nblk, P).T.astype(NP_BF16).view(np.int16)
            core_idx[c].append(np.concatenate([idx_t, seg_t, w_t], axis=1))

    edata = [np.ascontiguousarray(np.concatenate(core_idx[c], axis=1))
             for c in range(N_CORES)]
    return chunk_meta, edata


# ------------------------------------------------------------- device build --

def _build(metas, ecols):
    nc = bacc.Bacc(None, target_bir_lowering=False, num_devices=N_CORES,
                   num_swdge_queues=2)

    xT = nc.dram_tensor("xT", [IN_DIM, SHARD], BF16, kind="ExternalInput")
    w1T = nc.dram_tensor("w1T", [IN_DIM, HID], BF16, kind="ExternalInput")
    b1 = nc.dram_tensor("b1", [HID, 1], F32, kind="ExternalInput")
    w2T = nc.dram_tensor("w2T", [HID, HID], BF16, kind="ExternalInput")
    b2 = nc.dram_tensor("b2", [HID, 1], F32, kind="ExternalInput")
    conv_wT = nc.dram_tensor("conv_wT", [2, 2, HID, HID], BF16, kind="ExternalInput")
    conv_b = nc.dram_tensor("conv_b", [2, HID, 1], F32, kind="ExternalInput")
    headWT = nc.dram_tensor("headWT", [HID, 2 * LAT], BF16, kind="ExternalInput")
    head_b = nc.dram_tensor("head_b", [2 * LAT, 1], F32, kind="ExternalInput")
    iota = nc.dram_tensor("iota", [P, W, MAXBLK], BF16, kind="ExternalInput")
    edata = nc.dram_tensor("edata", [P, ecols], I16, kind="ExternalInput")
    muv_out = nc.dram_tensor("muvT", [2 * LAT, SHARD], F32, kind="ExternalOutput")

    ag_in = [nc.dram_tensor(f"ag_in{i}", [SHARD, HID], BF16) for i in range(2)]
    tables = [nc.dram_tensor(f"h_full{i}", [N_NODES, HID], BF16,
                             addr_space="Shared") for i in range(2)]

    chunk_ioff = []
    io = 0
    for m in metas:
        chunk_ioff.append(io)
        io += m["nblk"] * 10
    assert io == ecols

    with TileContext(nc) as tc:
        with (
            tc.tile_pool(name="const", bufs=1) as cp,
            tc.tile_pool(name="big", bufs=1) as bigp,
            tc.tile_pool(name="work", bufs=3) as wp,
            tc.tile_pool(name="msgp", bufs=2) as msgp,
            tc.tile_pool(name="swp", bufs=3) as swp,
            tc.tile_pool(name="ps_lin", bufs=3, space="PSUM") as ps_lin,
            tc.tile_pool(name="ps_tr", bufs=2, space="PSUM") as ps_tr,
            tc.tile_pool(name="ps_agg", bufs=3, space="PSUM") as ps_agg,
        ):
            # ---- constants ----
            w1t_sb = [cp.tile([P, HID], BF16, tag=f"w1_{k}", name=f"w1t_{k}") for k in range(4)]
            for k in range(4):
                nc.sync.dma_start(out=w1t_sb[k][:], in_=w1T[k * P:(k + 1) * P, :])
            w2t_sb = cp.tile([P, HID], BF16, tag="w2")
            nc.sync.dma_start(out=w2t_sb[:], in_=w2T[:, :])
            cw_sb = [[cp.tile([P, HID], BF16, tag=f"cw{l}{m}", name=f"cw_{l}_{m}") for m in range(2)]
                     for l in range(2)]
            for l in range(2):
                for m in range(2):
                    nc.scalar.dma_start(out=cw_sb[l][m][:], in_=conv_wT[l, m, :, :])
            b1_sb = cp.tile([P, 1], F32, tag="b1")
            nc.scalar.dma_start(out=b1_sb[:], in_=b1[:, :])
            b2_sb = cp.tile([P, 1], F32, tag="b2")
            nc.scalar.dma_start(out=b2_sb[:], in_=b2[:, :])
            cb_sb = [cp.tile([P, 1], F32, tag=f"cb{l}", name=f"cb_{l}") for l in range(2)]
            for l in range(2):
                nc.scalar.dma_start(out=cb_sb[l][:], in_=conv_b[l, :, :])
            hw_sb = cp.tile([P, 2 * LAT], BF16, tag="hw")
            nc.scalar.dma_start(out=hw_sb[:], in_=headWT[:, :])
            hb_sb = cp.tile([2 * LAT, 1], F32, tag="hb")
            nc.scalar.dma_start(out=hb_sb[:], in_=head_b[:, :])
            iota_sb = cp.tile([P, W, MAXBLK], BF16, tag="iota")
            nc.sync.dma_start(out=iota_sb[:], in_=iota[:, :, :])
            ident = cp.tile([P, P], F32, tag="ident")
            make_identity(nc, ident[:])
            identb = cp.tile([P, P], BF16, tag="identb")
            nc.vector.tensor_copy(out=identb[:], in_=ident[:])

            # all edge metadata (shared by both conv layers); loaded after the
            # MLP's x loads are queued
            edall = bigp.tile([P, ecols], I16, tag="edall")

            hA = bigp.tile([P, SHARD], BF16, tag="hA")   # h2T, then h4T
            hB = bigp.tile([P, SHARD], BF16, tag="hB")   # h3T
            aggT = bigp.tile([P, SHARD], BF16, tag="aggT")
            stage = bigp.tile([P, NB_STAGE * P], BF16, tag="stage")

            def emit_publish(hT_tile, t_idx, col, nt, eng=None):
                # transpose 128-col blocks into natural layout in `stage`,
                # then one batched DMA per tile into ag_in
                eng = eng or nc.sync
                b0 = col // P
                nfull = nt // P
                nblks = (nt + P - 1) // P
                for i in range(nblks):
                    w_ = min(P, nt - i * P)
                    tr_ps = ps_tr.tile([P, P], BF16, space="PSUM", tag="tr",
                                       name="trp")
                    nc.tensor.transpose(out=tr_ps[:w_, :],
                                        in_=hT_tile[:, col + i * P:col + i * P + w_],
                                        identity=identb[:])
                    nc.vector.tensor_copy(
                        out=stage[:w_, (b0 + i) * P:(b0 + i + 1) * P],
                        in_=tr_ps[:w_, :])
                if nfull:
                    eng.dma_start(
                        out=ag_in[t_idx][col:col + nfull * P, :].rearrange(
                            "(b p) f -> p b f", p=P),
                        in_=stage[:, col:col + nfull * P].rearrange(
                            "p (b f) -> p b f", f=P))
                if nt % P:
                    r0 = col + nfull * P
                    rb = b0 + nfull
                    eng.dma_start(
                        out=ag_in[t_idx][r0:r0 + nt % P, :],
                        in_=stage[:nt % P, rb * P:(rb + 1) * P])

            def emit_allgather(t_idx):
                nc.gpsimd.collective_compute(
                    "AllGather", mybir.AluOpType.bypass,
                    replica_groups=[list(range(N_CORES))],
                    ins=[ag_in[t_idx][:, :]],
                    outs=[tables[t_idx][:, :]],
                )

            def build_sw(ci):
                meta = metas[ci]
                nblk = meta["nblk"]
                io = chunk_ioff[ci]
                seg_t = edall[:, io + nblk * 8:io + nblk * 9].bitcast(BF16)
                w_t = edall[:, io + nblk * 9:io + nblk * 10].bitcast(BF16)
                s_w = swp.tile([P, W, MAXBLK], BF16, tag="sw")
                nc.vector.tensor_tensor(
                    out=s_w[:, :, :nblk],
                    in0=seg_t.unsqueeze(1).to_broadcast([P, W, nblk]),
                    in1=iota_sb[:, :, :nblk],
                    op=mybir.AluOpType.is_equal)
                nc.vector.tensor_tensor(
                    out=s_w[:, :, :nblk], in0=s_w[:, :, :nblk],
                    in1=w_t.unsqueeze(1).to_broadcast([P, W, nblk]),
                    op=mybir.AluOpType.mult)
                return s_w

            # ---- MLP (bf16 matmuls, f32 psum) ----
            xfp_cm = tc.tile_pool(name="xf", bufs=1)
            xfp = xfp_cm.__enter__()
            xfull = [xfp.tile([P, SHARD], BF16, tag=f"xf{k}", name=f"xf_{k}")
                     for k in range(4)]
            # column-chunked loads so the first MLP tile starts early, split
            # across the two HWDGE queues (SP and Activation)
            CCH = [(0, 1024), (1024, 2048), (2048, 3072), (3072, 4608),
                   (4608, SHARD)]
            for (c0, c1) in CCH:
                for k in range(4):
                    eng = nc.sync if k % 2 == 0 else nc.scalar
                    eng.dma_start(out=xfull[k][:, c0:c1],
                                  in_=xT[k * P:(k + 1) * P, c0:c1])
            col = 0
            for nt in N_TILES:
                h1_ps = ps_lin.tile([P, 512], F32, space="PSUM", tag="lin")
                for k in range(4):
                    nc.tensor.matmul(out=h1_ps[:, :nt], lhsT=w1t_sb[k][:],
                                     rhs=xfull[k][:, col:col + nt],
                                     start=(k == 0), stop=(k == 3))
                h1_sb = wp.tile([P, 512], BF16, tag="h1")
                nc.scalar.activation(out=h1_sb[:, :nt], in_=h1_ps[:, :nt],
                                     func=mybir.ActivationFunctionType.Relu,
                                     bias=b1_sb[:])
                h2_ps = ps_lin.tile([P, 512], F32, space="PSUM", tag="lin")
                nc.tensor.matmul(out=h2_ps[:, :nt], lhsT=w2t_sb[:],
                                 rhs=h1_sb[:, :nt], start=True, stop=True)
                nc.scalar.activation(out=hA[:, col:col + nt], in_=h2_ps[:, :nt],
                                     func=mybir.ActivationFunctionType.Relu,
                                     bias=b2_sb[:])
                emit_publish(hA, 0, col, nt)
                col += nt

            def conv_layer(layer, hT_in, hT_out, table, pub_idx=None,
                           heads=False, sw_cache=None):
                tiles_done = 0
                groups_done = 0

                def emit_linear(t):
                    col = 512 * t
                    nt = N_TILES[t]
                    ps = ps_lin.tile([P, 512], F32, space="PSUM", tag="lin")
                    nc.tensor.matmul(out=ps[:, :nt], lhsT=cw_sb[layer][0][:],
                                     rhs=aggT[:, col:col + nt],
                                     start=True, stop=False)
                    nc.tensor.matmul(out=ps[:, :nt], lhsT=cw_sb[layer][1][:],
                                     rhs=hT_in[:, col:col + nt],
                                     start=False, stop=True)
                    nc.scalar.activation(out=hT_out[:, col:col + nt],
                                         in_=ps[:, :nt],
                                         func=mybir.ActivationFunctionType.Relu,
                                         bias=cb_sb[layer][:])
                    if pub_idx is not None:
                        emit_publish(hT_out, pub_idx, col, nt)
                    if heads:
                        psh = ps_lin.tile([2 * LAT, 512], F32, space="PSUM",
                                          tag="lin", name="headps")
                        nc.tensor.matmul(out=psh[:, :nt], lhsT=hw_sb[:],
                                         rhs=hT_out[:, col:col + nt],
                                         start=True, stop=True)
                        muv = wp.tile([2 * LAT, 512], F32, tag="muv",
                                      name="muv")
                        nc.vector.tensor_tensor(
                            out=muv[:, :nt], in0=psh[:, :nt],
                            in1=hb_sb[:].to_broadcast([2 * LAT, nt]),
                            op=mybir.AluOpType.add)
                        nc.sync.dma_start(out=muv_out[:, col:col + nt],
                                          in_=muv[:, :nt])

                for ci, meta in enumerate(metas):
                    nblk, nblk_lo = meta["nblk"], meta["nblk_lo"]
                    io = chunk_ioff[ci]
                    idx_t = edall[:, io:io + nblk * 8]

                    msgL = msgp.tile([P, HCAP, HID], BF16, tag="msgL")
                    msgH = msgp.tile([P, HCAP, HID], BF16, tag="msgH")
                    if nblk_lo:
                        nc.gpsimd.dma_gather(
                            out_ap=msgL[:, :nblk_lo, :], in_ap=table[:HALF, :],
                            idxs_ap=idx_t[:, :nblk_lo * 8],
                            num_idxs=nblk_lo * P, num_idxs_reg=nblk_lo * P,
                            elem_size=HID, single_packet=False,
                            queue_num=0)
                    if nblk - nblk_lo:
                        nh = nblk - nblk_lo
                        nc.gpsimd.dma_gather(
                            out_ap=msgH[:, :nh, :], in_ap=table[HALF:, :],
                            idxs_ap=idx_t[:, nblk_lo * 8:nblk * 8],
                            num_idxs=nh * P, num_idxs_reg=nh * P,
                            elem_size=HID, single_packet=False,
                            queue_num=1)

                    if sw_cache is not None and ci in sw_cache:
                        s_w = sw_cache.pop(ci)
                    else:
                        s_w = build_sw(ci)

                    # one psum + one eviction per group: a group's lo and hi
                    # runs accumulate into the same tile
                    by_group = {}
                    for (g, h, b0, b1_) in meta["runs"]:
                        by_group.setdefault(g, []).append((h, b0, b1_))
                    for g in meta["groups"]:
                        ps = ps_agg.tile([P, W], F32, space="PSUM", tag="agg")
                        blocks = [(h, b) for (h, b0, b1_) in by_group[g]
                                  for b in range(b0, b1_)]
                        for i, (h, b) in enumerate(blocks):
                            mt = msgL[:, b, :] if h == 0 else \
                                 msgH[:, b - nblk_lo, :]
                            nc.tensor.matmul(out=ps[:], lhsT=mt,
                                             rhs=s_w[:, :, b],
                                             start=(i == 0),
                                             stop=(i == len(blocks) - 1))
                        gw = min(W, SHARD - g * W)
                        nc.scalar.activation(
                            out=aggT[:, g * W:g * W + gw], in_=ps[:, :gw],
                            func=mybir.ActivationFunctionType.Copy)
                    groups_done += len(meta["groups"])

                    while (tiles_done < len(N_TILES)
                           and groups_done * W >= tiles_done * 512
                           + N_TILES[tiles_done]):
                        emit_linear(tiles_done)
                        tiles_done += 1
                while tiles_done < len(N_TILES):
                    emit_linear(tiles_done)
                    tiles_done += 1

            xfp_cm.__exit__(None, None, None)
            nc.scalar.dma_start(out=edall[:], in_=edata[:, :])

            # prebuild the first S_w tiles while the AllGather is in flight
            sw_cache = {ci: build_sw(ci) for ci in range(min(3, len(metas)))}
            emit_allgather(0)
            conv_layer(0, hA, hB, tables[0], pub_idx=1, sw_cache=sw_cache)
            sw_cache = {ci: build_sw(ci) for ci in range(min(3, len(metas)))}
            emit_allgather(1)
            conv_layer(1, hB, hA, tables[1], heads=True, sw_cache=sw_cache)

    nc.finalize()
    return nc


# -------------------------------------------------------------------- driver --

def _get_compiled(x, edge_index, edge_attr, weights):
    src = np.asarray(edge_index[0]).astype(np.int64)
    dst = np.asarray(edge_index[1]).astype(np.int64)
    wgt = np.asarray(edge_attr, dtype=np.float32)
    x = np.asarray(x, dtype=np.float32)

    per_core_edges = []
    for c in range(N_CORES):
        sel = (dst >= c * SHARD) & (dst < (c + 1) * SHARD)
        s, d, wv = src[sel], dst[sel] - c * SHARD, wgt[sel]
        order = np.argsort(d, kind="stable")
        per_core_edges.append((s[order], d[order], wv[order]))

    metas, edata = _unified_structure(per_core_edges)
    ecols = sum(m["nblk"] * 10 for m in metas)

    nc = _build(metas, ecols)

    (W1, b1, W2, b2, g1_rel_W, g1_rel_b, g1_root_W,
     g2_rel_W, g2_rel_b, g2_root_W, mu_W, mu_b, lv_W, lv_b) = [
        np.asarray(w, dtype=np.float32) for w in weights]

    conv_wT = np.stack([
        np.stack([g1_rel_W.T, g1_root_W.T]),
        np.stack([g2_rel_W.T, g2_root_W.T]),
    ]).astype(NP_BF16).copy()
    conv_b = np.stack([g1_rel_b[:, None], g2_rel_b[:, None]]).copy()
    headWT = np.ascontiguousarray(
        np.concatenate([mu_W, lv_W], axis=0).T.astype(NP_BF16))
    head_b = np.concatenate([mu_b, lv_b])[:, None].copy()
    iota = np.ascontiguousarray(np.broadcast_to(
        np.arange(W, dtype=np.float32)[None, :, None],
        (P, W, MAXBLK)).astype(NP_BF16))

    common = dict(
        w1T=np.ascontiguousarray(W1.T.astype(NP_BF16)), b1=b1[:, None].copy(),
        w2T=np.ascontiguousarray(W2.T.astype(NP_BF16)), b2=b2[:, None].copy(),
        conv_wT=conv_wT, conv_b=conv_b, headWT=headWT, head_b=head_b,
        iota=iota,
    )
    in_maps = []
    for c in range(N_CORES):
        m = dict(common)
        m["xT"] = np.ascontiguousarray(x[c * SHARD:(c + 1) * SHARD, :].T.astype(NP_BF16))
        m["edata"] = edata[c]
        in_maps.append(m)
    return nc, in_maps


def kernel(x, edge_index, edge_attr,
           W1, b1, W2, b2,
           g1_rel_W, g1_rel_b, g1_root_W,
           g2_rel_W, g2_rel_b, g2_root_W,
           mu_W, mu_b, lv_W, lv_b):
    weights = (W1, b1, W2, b2, g1_rel_W, g1_rel_b, g1_root_W,
               g2_rel_W, g2_rel_b, g2_root_W, mu_W, mu_b, lv_W, lv_b)
    nc, in_maps = _get_compiled(x, edge_index, edge_attr, weights)
    res = bass_utils.run_bass_kernel_spmd(nc, in_maps,
                                          core_ids=list(range(N_CORES)))
    muvT = np.concatenate([res.results[c]["muvT"] for c in range(N_CORES)],
                          axis=1)
    return (np.ascontiguousarray(muvT[:LAT, :].T),
            np.ascontiguousarray(muvT[LAT:, :].T))


# revision 20
# speedup vs baseline: 1.1332x; 1.0024x over previous
"""DRASI encoder (MLP -> GraphConv x2 -> mu/logvar heads) on 8 Trainium2 cores.

Sharding: nodes are split into 8 contiguous shards of 6250. Each core runs the
node-local MLP on its shard (transposed layout, weights as matmul lhsT), the
shards are AllGathered into a full [50000, 128] feature table in DRAM, and
each core processes the edges whose destination lies in its shard:

  - edges are sorted by dst and bucketed into 64-node "groups"; each
    (group, src-half) bucket is padded to whole 128-edge blocks, with the
    block count unified across cores (max) so all 8 cores share one program;
  - dma_gather fetches source rows from the table (int16 indices, so the
    table is addressed as two 25000-row halves);
  - a DVE-built selection matrix S_w[e, s] = w_e * (seg_e == s) turns the
    segment sum into per-block PE matmuls accumulating aggT = msg.T @ S_w
    in PSUM (features x group-nodes), evicted per bucket into an SBUF table;
  - the GraphConv linear layers + relu run on the transposed activations
    (all bf16 operands so the PE runs at full rate), interleaved per
    512-column tile with the remaining gathers; publishes go through a
    natural-layout staging tile and batched DMAs.

Outputs (mu, logvar) are computed per shard and concatenated on the host.
"""
import sys
sys.path.insert(0, '/opt/trn_rl_repo')

import numpy as np
import concourse.bass as bass
import concourse.bacc as bacc
import concourse.mybir as mybir
from concourse.tile import TileContext
from concourse.masks import make_identity
from concourse import bass_utils

P = 128
N_CORES = 8
N_NODES = 50000
IN_DIM = 512
HID = 128
LAT = 32
SHARD = N_NODES // N_CORES          # 6250
HALF = N_NODES // 2                 # 25000
W = 64                              # nodes per segment group (PSUM tile width)
MAXBLK = 48                         # max 128-edge blocks per gather chunk
HCAP = 28                           # max blocks per src-half within a chunk
N_GROUPS = (SHARD + W - 1) // W     # 98
N_TILES = [512] * (SHARD // 512) + ([SHARD % 512] if SHARD % 512 else [])
NB_STAGE = (SHARD + P - 1) // P     # 49 row-blocks in the staging tile
F32 = mybir.dt.float32
BF16 = mybir.dt.bfloat16
I16 = mybir.dt.int16
import ml_dtypes
NP_BF16 = ml_dtypes.bfloat16


# ---------------------------------------------------------------- host prep --

def _unified_structure(per_core_edges):
    """per_core_edges: list of (src, dst_local, w) sorted by dst_local.
    Returns (chunk_meta, per-core device arrays eidx/eseg/ew)."""
    # bucket edges per core into (group, half)
    buckets = [[[None, None] for _ in range(N_GROUPS)] for _ in range(N_CORES)]
    for c, (src, dstl, wgt) in enumerate(per_core_edges):
        grp = dstl // W
        for g in range(N_GROUPS):
            sel = grp == g
            gs, gd, gw = src[sel], dstl[sel], wgt[sel]
            hi = gs >= HALF
            for h in (0, 1):
                m = hi == bool(h)
                buckets[c][g][h] = (gs[m] - h * HALF, gd[m] - g * W, gw[m])

    # unified block count per (group, half): max over cores, >= 1 block per
    # group total so every group gets an eviction
    B = np.zeros((N_GROUPS, 2), np.int64)
    for g in range(N_GROUPS):
        for h in (0, 1):
            B[g, h] = max((buckets[c][g][h][0].shape[0] + P - 1) // P
                          for c in range(N_CORES))
        if B[g, 0] == 0 and B[g, 1] == 0:
            B[g, 0] = 1

    # pack consecutive groups into chunks of <= MAXBLK blocks, with each
    # src-half capped at HCAP (separate msgL/msgH tiles); the last group is
    # kept as its own small chunk so the window's serial tail after the final
    # gather is short
    chunks = []
    cur, cur_lo, cur_hi = [], 0, 0
    for g in range(N_GROUPS - 1):
        lo, hi = int(B[g, 0]), int(B[g, 1])
        if cur and (cur_lo + lo > HCAP or cur_hi + hi > HCAP
                    or cur_lo + cur_hi + lo + hi > MAXBLK):
            chunks.append(cur)
            cur, cur_lo, cur_hi = [], 0, 0
        cur.append(g)
        cur_lo += lo
        cur_hi += hi
    if cur:
        chunks.append(cur)
    chunks.append([N_GROUPS - 1])

    chunk_meta = []
    core_idx = [[] for _ in range(N_CORES)]
    for groups in chunks:
        nblk_lo = int(sum(B[g, 0] for g in groups))
        nblk_hi = int(sum(B[g, 1] for g in groups))
        nblk = nblk_lo + nblk_hi
        runs = []
        b = 0
        for h in (0, 1):
            for g in groups:
                nb = int(B[g, h])
                if nb:
                    runs.append((g, h, b, b + nb))
                    b += nb
        chunk_meta.append(dict(nblk=nblk, nblk_lo=nblk_lo, runs=runs,
                               groups=list(groups)))

        for c in range(N_CORES):
            idx_flat = np.zeros(nblk * P, np.int16)
            seg_flat = np.zeros(nblk * P, np.float32)
            w_flat = np.zeros(nblk * P, np.float32)
            for (g, h, b0, b1_) in runs:
                ids, segs, ws = buckets[c][g][h]
                n = ids.shape[0]
                o = b0 * P
                idx_flat[o:o + n] = ids.astype(np.int16)
                seg_flat[o:o + n] = segs.astype(np.float32)
                w_flat[o:o + n] = ws
            idx_t = np.tile(idx_flat.reshape(nblk * 8, 16).T, (8, 1))
            seg_t = seg_flat.reshape(nblk, P).T.astype(NP_BF16).view(np.int16)
            w_t = w_flat.reshape(nblk, P).T.astype(NP_BF16).view(np.int16)
            core_idx[c].append(np.concatenate([idx_t, seg_t, w_t], axis=1))

    edata = [np.ascontiguousarray(np.concatenate(core_idx[c], axis=1))
             for c in range(N_CORES)]
    return chunk_meta, edata


# ------------------------------------------------------------- device build --

def _build(metas, ecols):
    nc = bacc.Bacc(None, target_bir_lowering=False, num_devices=N_CORES,
                   num_swdge_queues=2)

    xT = nc.dram_tensor("xT", [IN_DIM, SHARD], BF16, kind="ExternalInput")
    w1T = nc.dram_tensor("w1T", [IN_DIM, HID], BF16, kind="ExternalInput")
    b1 = nc.dram_tensor("b1", [HID, 1], F32, kind="ExternalInput")
    w2T = nc.dram_tensor("w2T", [HID, HID], BF16, kind="ExternalInput")
    b2 = nc.dram_tensor("b2", [HID, 1], F32, kind="ExternalInput")
    conv_wT = nc.dram_tensor("conv_wT", [2, 2, HID, HID], BF16, kind="ExternalInput")
    conv_b = nc.dram_tensor("conv_b", [2, HID, 1], F32, kind="ExternalInput")
    headWT = nc.dram_tensor("headWT", [HID, 2 * LAT], BF16, kind="ExternalInput")
    head_b = nc.dram_tensor("head_b", [2 * LAT, 1], F32, kind="ExternalInput")
    iota = nc.dram_tensor("iota", [P, W, MAXBLK], BF16, kind="ExternalInput")
    edata = nc.dram_tensor("edata", [P, ecols], I16, kind="ExternalInput")
    muv_out = nc.dram_tensor("muvT", [2 * LAT, SHARD], F32, kind="ExternalOutput")

    ag_in = [nc.dram_tensor(f"ag_in{i}", [SHARD, HID], BF16) for i in range(2)]
    tables = [nc.dram_tensor(f"h_full{i}", [N_NODES, HID], BF16,
                             addr_space="Shared") for i in range(2)]

    chunk_ioff = []
    io = 0
    for m in metas:
        chunk_ioff.append(io)
        io += m["nblk"] * 10
    assert io == ecols

    with TileContext(nc) as tc:
        with (
            tc.tile_pool(name="const", bufs=1) as cp,
            tc.tile_pool(name="big", bufs=1) as bigp,
            tc.tile_pool(name="work", bufs=3) as wp,
            tc.tile_pool(name="msgp", bufs=2) as msgp,
            tc.tile_pool(name="swp", bufs=3) as swp,
            tc.tile_pool(name="ps_lin", bufs=3, space="PSUM") as ps_lin,
            tc.tile_pool(name="ps_tr", bufs=2, space="PSUM") as ps_tr,
            tc.tile_pool(name="ps_agg", bufs=3, space="PSUM") as ps_agg,
        ):
            # ---- constants ----
            w1t_sb = [cp.tile([P, HID], BF16, tag=f"w1_{k}", name=f"w1t_{k}") for k in range(4)]
            for k in range(4):
                nc.sync.dma_start(out=w1t_sb[k][:], in_=w1T[k * P:(k + 1) * P, :])
            w2t_sb = cp.tile([P, HID], BF16, tag="w2")
            nc.sync.dma_start(out=w2t_sb[:], in_=w2T[:, :])
            cw_sb = [[cp.tile([P, HID], BF16, tag=f"cw{l}{m}", name=f"cw_{l}_{m}") for m in range(2)]
                     for l in range(2)]
            for l in range(2):
                for m in range(2):
                    nc.scalar.dma_start(out=cw_sb[l][m][:], in_=conv_wT[l, m, :, :])
            b1_sb = cp.tile([P, 1], F32, tag="b1")
            nc.scalar.dma_start(out=b1_sb[:], in_=b1[:, :])
            b2_sb = cp.tile([P, 1], F32, tag="b2")
            nc.scalar.dma_start(out=b2_sb[:], in_=b2[:, :])
            cb_sb = [cp.tile([P, 1], F32, tag=f"cb{l}", name=f"cb_{l}") for l in range(2)]
            for l in range(2):
                nc.scalar.dma_start(out=cb_sb[l][:], in_=conv_b[l, :, :])
            hw_sb = cp.tile([P, 2 * LAT], BF16, tag="hw")
            nc.scalar.dma_start(out=hw_sb[:], in_=headWT[:, :])
            hb_sb = cp.tile([2 * LAT, 1], F32, tag="hb")
            nc.scalar.dma_start(out=hb_sb[:], in_=head_b[:, :])
            iota_sb = cp.tile([P, W, MAXBLK], BF16, tag="iota")
            nc.sync.dma_start(out=iota_sb[:], in_=iota[:, :, :])
            ident = cp.tile([P, P], F32, tag="ident")
            make_identity(nc, ident[:])
            identb = cp.tile([P, P], BF16, tag="identb")
            nc.vector.tensor_copy(out=identb[:], in_=ident[:])

            # all edge metadata (shared by both conv layers); loaded after the
            # MLP's x loads are queued
            edall = bigp.tile([P, ecols], I16, tag="edall")

            hA = bigp.tile([P, SHARD], BF16, tag="hA")   # h2T, then h4T
            hB = bigp.tile([P, SHARD], BF16, tag="hB")   # h3T
            aggT = bigp.tile([P, SHARD], BF16, tag="aggT")
            stage = bigp.tile([P, NB_STAGE * P], BF16, tag="stage")

            def emit_publish(hT_tile, t_idx, col, nt, eng=None):
                # transpose 128-col blocks into natural layout in `stage`,
                # then one batched DMA per tile into ag_in
                eng = eng or nc.sync
                b0 = col // P
                nfull = nt // P
                nblks = (nt + P - 1) // P
                for i in range(nblks):
                    w_ = min(P, nt - i * P)
                    tr_ps = ps_tr.tile([P, P], BF16, space="PSUM", tag="tr",
                                       name="trp")
                    nc.tensor.transpose(out=tr_ps[:w_, :],
                                        in_=hT_tile[:, col + i * P:col + i * P + w_],
                                        identity=identb[:])
                    nc.vector.tensor_copy(
                        out=stage[:w_, (b0 + i) * P:(b0 + i + 1) * P],
                        in_=tr_ps[:w_, :])
                if nfull:
                    eng.dma_start(
                        out=ag_in[t_idx][col:col + nfull * P, :].rearrange(
                            "(b p) f -> p b f", p=P),
                        in_=stage[:, col:col + nfull * P].rearrange(
                            "p (b f) -> p b f", f=P))
                if nt % P:
                    r0 = col + nfull * P
                    rb = b0 + nfull
                    eng.dma_start(
                        out=ag_in[t_idx][r0:r0 + nt % P, :],
                        in_=stage[:nt % P, rb * P:(rb + 1) * P])

            def emit_allgather(t_idx):
                nc.gpsimd.collective_compute(
                    "AllGather", mybir.AluOpType.bypass,
                    replica_groups=[list(range(N_CORES))],
                    ins=[ag_in[t_idx][:, :]],
                    outs=[tables[t_idx][:, :]],
                )

            def build_sw(ci):
                meta = metas[ci]
                nblk = meta["nblk"]
                io = chunk_ioff[ci]
                seg_t = edall[:, io + nblk * 8:io + nblk * 9].bitcast(BF16)
                w_t = edall[:, io + nblk * 9:io + nblk * 10].bitcast(BF16)
                s_w = swp.tile([P, W, MAXBLK], BF16, tag="sw")
                nc.vector.tensor_tensor(
                    out=s_w[:, :, :nblk],
                    in0=seg_t.unsqueeze(1).to_broadcast([P, W, nblk]),
                    in1=iota_sb[:, :, :nblk],
                    op=mybir.AluOpType.is_equal)
                nc.vector.tensor_tensor(
                    out=s_w[:, :, :nblk], in0=s_w[:, :, :nblk],
                    in1=w_t.unsqueeze(1).to_broadcast([P, W, nblk]),
                    op=mybir.AluOpType.mult)
                return s_w

            # ---- MLP (bf16 matmuls, f32 psum) ----
            xfp_cm = tc.tile_pool(name="xf", bufs=1)
            xfp = xfp_cm.__enter__()
            xfull = [xfp.tile([P, SHARD], BF16, tag=f"xf{k}", name=f"xf_{k}")
                     for k in range(4)]
            # column-chunked loads so the first MLP tile starts early, split
            # across the two HWDGE queues (SP and Activation)
            CCH = [(0, 1024), (1024, 2048), (2048, 3072), (3072, 4608),
                   (4608, SHARD)]
            for (c0, c1) in CCH:
                for k in range(4):
                    eng = nc.sync if k % 2 == 0 else nc.scalar
                    eng.dma_start(out=xfull[k][:, c0:c1],
                                  in_=xT[k * P:(k + 1) * P, c0:c1])
            col = 0
            for nt in N_TILES:
                h1_ps = ps_lin.tile([P, 512], F32, space="PSUM", tag="lin")
                for k in range(4):
                    nc.tensor.matmul(out=h1_ps[:, :nt], lhsT=w1t_sb[k][:],
                                     rhs=xfull[k][:, col:col + nt],
                                     start=(k == 0), stop=(k == 3))
                h1_sb = wp.tile([P, 512], BF16, tag="h1")
                # relu(psum + b1) as one fused DVE op, freeing the Activation
                # engine's queue for the x-load DMAs
                nc.vector.tensor_scalar(
                    out=h1_sb[:, :nt], in0=h1_ps[:, :nt], scalar1=b1_sb[:],
                    scalar2=0.0, op0=mybir.AluOpType.add,
                    op1=mybir.AluOpType.max)
                h2_ps = ps_lin.tile([P, 512], F32, space="PSUM", tag="lin")
                nc.tensor.matmul(out=h2_ps[:, :nt], lhsT=w2t_sb[:],
                                 rhs=h1_sb[:, :nt], start=True, stop=True)
                nc.scalar.activation(out=hA[:, col:col + nt], in_=h2_ps[:, :nt],
                                     func=mybir.ActivationFunctionType.Relu,
                                     bias=b2_sb[:])
                emit_publish(hA, 0, col, nt)
                col += nt

            def conv_layer(layer, hT_in, hT_out, table, pub_idx=None,
                           heads=False, sw_cache=None):
                tiles_done = 0
                groups_done = 0

                def emit_linear(t):
                    col = 512 * t
                    nt = N_TILES[t]
                    ps = ps_lin.tile([P, 512], F32, space="PSUM", tag="lin")
                    nc.tensor.matmul(out=ps[:, :nt], lhsT=cw_sb[layer][0][:],
                                     rhs=aggT[:, col:col + nt],
                                     start=True, stop=False)
                    nc.tensor.matmul(out=ps[:, :nt], lhsT=cw_sb[layer][1][:],
                                     rhs=hT_in[:, col:col + nt],
                                     start=False, stop=True)
                    nc.scalar.activation(out=hT_out[:, col:col + nt],
                                         in_=ps[:, :nt],
                                         func=mybir.ActivationFunctionType.Relu,
                                         bias=cb_sb[layer][:])
                    if pub_idx is not None:
                        emit_publish(hT_out, pub_idx, col, nt)
                    if heads:
                        psh = ps_lin.tile([2 * LAT, 512], F32, space="PSUM",
                                          tag="lin", name="headps")
                        nc.tensor.matmul(out=psh[:, :nt], lhsT=hw_sb[:],
                                         rhs=hT_out[:, col:col + nt],
                                         start=True, stop=True)
                        muv = wp.tile([2 * LAT, 512], F32, tag="muv",
                                      name="muv")
                        nc.vector.tensor_tensor(
                            out=muv[:, :nt], in0=psh[:, :nt],
                            in1=hb_sb[:].to_broadcast([2 * LAT, nt]),
                            op=mybir.AluOpType.add)
                        nc.sync.dma_start(out=muv_out[:, col:col + nt],
                                          in_=muv[:, :nt])

                for ci, meta in enumerate(metas):
                    nblk, nblk_lo = meta["nblk"], meta["nblk_lo"]
                    io = chunk_ioff[ci]
                    idx_t = edall[:, io:io + nblk * 8]

                    msgL = msgp.tile([P, HCAP, HID], BF16, tag="msgL")
                    msgH = msgp.tile([P, HCAP, HID], BF16, tag="msgH")
                    if nblk_lo:
                        nc.gpsimd.dma_gather(
                            out_ap=msgL[:, :nblk_lo, :], in_ap=table[:HALF, :],
                            idxs_ap=idx_t[:, :nblk_lo * 8],
                            num_idxs=nblk_lo * P, num_idxs_reg=nblk_lo * P,
                            elem_size=HID, single_packet=False,
                            queue_num=0)
                    if nblk - nblk_lo:
                        nh = nblk - nblk_lo
                        nc.gpsimd.dma_gather(
                            out_ap=msgH[:, :nh, :], in_ap=table[HALF:, :],
                            idxs_ap=idx_t[:, nblk_lo * 8:nblk * 8],
                            num_idxs=nh * P, num_idxs_reg=nh * P,
                            elem_size=HID, single_packet=False,
                            queue_num=1)

                    if sw_cache is not None and ci in sw_cache:
                        s_w = sw_cache.pop(ci)
                    else:
                        s_w = build_sw(ci)

                    # one psum + one eviction per group: a group's lo and hi
                    # runs accumulate into the same tile
                    by_group = {}
                    for (g, h, b0, b1_) in meta["runs"]:
                        by_group.setdefault(g, []).append((h, b0, b1_))
                    for g in meta["groups"]:
                        ps = ps_agg.tile([P, W], F32, space="PSUM", tag="agg")
                        blocks = [(h, b) for (h, b0, b1_) in by_group[g]
                                  for b in range(b0, b1_)]
                        for i, (h, b) in enumerate(blocks):
                            mt = msgL[:, b, :] if h == 0 else \
                                 msgH[:, b - nblk_lo, :]
                            nc.tensor.matmul(out=ps[:], lhsT=mt,
                                             rhs=s_w[:, :, b],
                                             start=(i == 0),
                                             stop=(i == len(blocks) - 1))
                        gw = min(W, SHARD - g * W)
                        nc.scalar.activation(
                            out=aggT[:, g * W:g * W + gw], in_=ps[:, :gw],
                            func=mybir.ActivationFunctionType.Copy)
                    groups_done += len(meta["groups"])

                    while (tiles_done < len(N_TILES)
                           and groups_done * W >= tiles_done * 512
                           + N_TILES[tiles_done]):
                        emit_linear(tiles_done)
                        tiles_done += 1
                while tiles_done < len(N_TILES):
                    emit_linear(tiles_done)
                    tiles_done += 1

            xfp_cm.__exit__(None, None, None)
            nc.scalar.dma_start(out=edall[:], in_=edata[:, :])

            # prebuild the first S_w tiles while the AllGather is in flight
            sw_cache = {ci: build_sw(ci) for ci in range(min(3, len(metas)))}
            emit_allgather(0)
            conv_layer(0, hA, hB, tables[0], pub_idx=1, sw_cache=sw_cache)
            sw_cache = {ci: build_sw(ci) for ci in range(min(3, len(metas)))}
            emit_allgather(1)
            conv_layer(1, hB, hA, tables[1], heads=True, sw_cache=sw_cache)

    nc.finalize()
    return nc


# -------------------------------------------------------------------- driver --

def _get_compiled(x, edge_index, edge_attr, weights):
    src = np.asarray(edge_index[0]).astype(np.int64)
    dst = np.asarray(edge_index[1]).astype(np.int64)
    wgt = np.asarray(edge_attr, dtype=np.float32)
    x = np.asarray(x, dtype=np.float32)

    per_core_edges = []
    for c in range(N_CORES):
        sel = (dst >= c * SHARD) & (dst < (c + 1) * SHARD)
        s, d, wv = src[sel], dst[sel] - c * SHARD, wgt[sel]
        order = np.argsort(d, kind="stable")
        per_core_edges.append((s[order], d[order], wv[order]))

    metas, edata = _unified_structure(per_core_edges)
    ecols = sum(m["nblk"] * 10 for m in metas)

    nc = _build(metas, ecols)

    (W1, b1, W2, b2, g1_rel_W, g1_rel_b, g1_root_W,
     g2_rel_W, g2_rel_b, g2_root_W, mu_W, mu_b, lv_W, lv_b) = [
        np.asarray(w, dtype=np.float32) for w in weights]

    conv_wT = np.stack([
        np.stack([g1_rel_W.T, g1_root_W.T]),
        np.stack([g2_rel_W.T, g2_root_W.T]),
    ]).astype(NP_BF16).copy()
    conv_b = np.stack([g1_rel_b[:, None], g2_rel_b[:, None]]).copy()
    headWT = np.ascontiguousarray(
        np.concatenate([mu_W, lv_W], axis=0).T.astype(NP_BF16))
    head_b = np.concatenate([mu_b, lv_b])[:, None].copy()
    iota = np.ascontiguousarray(np.broadcast_to(
        np.arange(W, dtype=np.float32)[None, :, None],
        (P, W, MAXBLK)).astype(NP_BF16))

    common = dict(
        w1T=np.ascontiguousarray(W1.T.astype(NP_BF16)), b1=b1[:, None].copy(),
        w2T=np.ascontiguousarray(W2.T.astype(NP_BF16)), b2=b2[:, None].copy(),
        conv_wT=conv_wT, conv_b=conv_b, headWT=headWT, head_b=head_b,
        iota=iota,
    )
    in_maps = []
    for c in range(N_CORES):
        m = dict(common)
        m["xT"] = np.ascontiguousarray(x[c * SHARD:(c + 1) * SHARD, :].T.astype(NP_BF16))
        m["edata"] = edata[c]
        in_maps.append(m)
    return nc, in_maps


def kernel(x, edge_index, edge_attr,
           W1, b1, W2, b2,
           g1_rel_W, g1_rel_b, g1_root_W,
           g2_rel_W, g2_rel_b, g2_root_W,
           mu_W, mu_b, lv_W, lv_b):
    weights = (W1, b1, W2, b2, g1_rel_W, g1_rel_b, g1_root_W,
               g2_rel_W, g2_rel_b, g2_root_W, mu_W, mu_b, lv_W, lv_b)
    nc, in_maps = _get_compiled(x, edge_index, edge_attr, weights)
    res = bass_utils.run_bass_kernel_spmd(nc, in_maps,
                                          core_ids=list(range(N_CORES)))
    muvT = np.concatenate([res.results[c]["muvT"] for c in range(N_CORES)],
                          axis=1)
    return (np.ascontiguousarray(muvT[:LAT, :].T),
            np.ascontiguousarray(muvT[LAT:, :].T))


# revision 21
# speedup vs baseline: 1.1336x; 1.0003x over previous
"""DRASI encoder (MLP -> GraphConv x2 -> mu/logvar heads) on 8 Trainium2 cores.

Sharding: nodes are split into 8 contiguous shards of 6250. Each core runs the
node-local MLP on its shard (transposed layout, weights as matmul lhsT), the
shards are AllGathered into a full [50000, 128] feature table in DRAM, and
each core processes the edges whose destination lies in its shard:

  - edges are sorted by dst and bucketed into 64-node "groups"; each
    (group, src-half) bucket is padded to whole 128-edge blocks, with the
    block count unified across cores (max) so all 8 cores share one program;
  - dma_gather fetches source rows from the table (int16 indices, so the
    table is addressed as two 25000-row halves);
  - a DVE-built selection matrix S_w[e, s] = w_e * (seg_e == s) turns the
    segment sum into per-block PE matmuls accumulating aggT = msg.T @ S_w
    in PSUM (features x group-nodes), evicted per bucket into an SBUF table;
  - the GraphConv linear layers + relu run on the transposed activations
    (all bf16 operands so the PE runs at full rate), interleaved per
    512-column tile with the remaining gathers; publishes go through a
    natural-layout staging tile and batched DMAs.

Outputs (mu, logvar) are computed per shard and concatenated on the host.
"""
import sys
sys.path.insert(0, '/opt/trn_rl_repo')

import numpy as np
import concourse.bass as bass
import concourse.bacc as bacc
import concourse.mybir as mybir
from concourse.tile import TileContext
from concourse.masks import make_identity
from concourse import bass_utils

P = 128
N_CORES = 8
N_NODES = 50000
IN_DIM = 512
HID = 128
LAT = 32
SHARD = N_NODES // N_CORES          # 6250
HALF = N_NODES // 2                 # 25000
W = 64                              # nodes per segment group (PSUM tile width)
MAXBLK = 48                         # max 128-edge blocks per gather chunk
HCAP = 28                           # max blocks per src-half within a chunk
N_GROUPS = (SHARD + W - 1) // W     # 98
N_TILES = [512] * (SHARD // 512) + ([SHARD % 512] if SHARD % 512 else [])
NB_STAGE = (SHARD + P - 1) // P     # 49 row-blocks in the staging tile
F32 = mybir.dt.float32
BF16 = mybir.dt.bfloat16
I16 = mybir.dt.int16
import ml_dtypes
NP_BF16 = ml_dtypes.bfloat16


# ---------------------------------------------------------------- host prep --

def _unified_structure(per_core_edges):
    """per_core_edges: list of (src, dst_local, w) sorted by dst_local.
    Returns (chunk_meta, per-core device arrays eidx/eseg/ew)."""
    # bucket edges per core into (group, half)
    buckets = [[[None, None] for _ in range(N_GROUPS)] for _ in range(N_CORES)]
    for c, (src, dstl, wgt) in enumerate(per_core_edges):
        grp = dstl // W
        for g in range(N_GROUPS):
            sel = grp == g
            gs, gd, gw = src[sel], dstl[sel], wgt[sel]
            hi = gs >= HALF
            for h in (0, 1):
                m = hi == bool(h)
                buckets[c][g][h] = (gs[m] - h * HALF, gd[m] - g * W, gw[m])

    # unified block count per (group, half): max over cores, >= 1 block per
    # group total so every group gets an eviction
    B = np.zeros((N_GROUPS, 2), np.int64)
    for g in range(N_GROUPS):
        for h in (0, 1):
            B[g, h] = max((buckets[c][g][h][0].shape[0] + P - 1) // P
                          for c in range(N_CORES))
        if B[g, 0] == 0 and B[g, 1] == 0:
            B[g, 0] = 1

    # pack consecutive groups into chunks of <= MAXBLK blocks, with each
    # src-half capped at HCAP (separate msgL/msgH tiles); the last group is
    # kept as its own small chunk so the window's serial tail after the final
    # gather is short
    chunks = []
    cur, cur_lo, cur_hi = [], 0, 0
    for g in range(N_GROUPS - 1):
        lo, hi = int(B[g, 0]), int(B[g, 1])
        if cur and (cur_lo + lo > HCAP or cur_hi + hi > HCAP
                    or cur_lo + cur_hi + lo + hi > MAXBLK):
            chunks.append(cur)
            cur, cur_lo, cur_hi = [], 0, 0
        cur.append(g)
        cur_lo += lo
        cur_hi += hi
    if cur:
        chunks.append(cur)
    chunks.append([N_GROUPS - 1])

    chunk_meta = []
    core_idx = [[] for _ in range(N_CORES)]
    for groups in chunks:
        nblk_lo = int(sum(B[g, 0] for g in groups))
        nblk_hi = int(sum(B[g, 1] for g in groups))
        nblk = nblk_lo + nblk_hi
        runs = []
        b = 0
        for h in (0, 1):
            for g in groups:
                nb = int(B[g, h])
                if nb:
                    runs.append((g, h, b, b + nb))
                    b += nb
        chunk_meta.append(dict(nblk=nblk, nblk_lo=nblk_lo, runs=runs,
                               groups=list(groups)))

        for c in range(N_CORES):
            idx_flat = np.zeros(nblk * P, np.int16)
            seg_flat = np.zeros(nblk * P, np.float32)
            w_flat = np.zeros(nblk * P, np.float32)
            for (g, h, b0, b1_) in runs:
                ids, segs, ws = buckets[c][g][h]
                n = ids.shape[0]
                o = b0 * P
                idx_flat[o:o + n] = ids.astype(np.int16)
                seg_flat[o:o + n] = segs.astype(np.float32)
                w_flat[o:o + n] = ws
            idx_t = np.tile(idx_flat.reshape(nblk * 8, 16).T, (8, 1))
            seg_t = seg_flat.reshape(nblk, P).T.astype(NP_BF16).view(np.int16)
            w_t = w_flat.reshape(nblk, P).T.astype(NP_BF16).view(np.int16)
            core_idx[c].append(np.concatenate([idx_t, seg_t, w_t], axis=1))

    edata = [np.ascontiguousarray(np.concatenate(core_idx[c], axis=1))
             for c in range(N_CORES)]
    return chunk_meta, edata


# ------------------------------------------------------------- device build --

def _build(metas, ecols):
    nc = bacc.Bacc(None, target_bir_lowering=False, num_devices=N_CORES,
                   num_swdge_queues=2)

    xT = nc.dram_tensor("xT", [IN_DIM, SHARD], BF16, kind="ExternalInput")
    w1T = nc.dram_tensor("w1T", [IN_DIM, HID], BF16, kind="ExternalInput")
    b1 = nc.dram_tensor("b1", [HID, 1], F32, kind="ExternalInput")
    w2T = nc.dram_tensor("w2T", [HID, HID], BF16, kind="ExternalInput")
    b2 = nc.dram_tensor("b2", [HID, 1], F32, kind="ExternalInput")
    conv_wT = nc.dram_tensor("conv_wT", [2, 2, HID, HID], BF16, kind="ExternalInput")
    conv_b = nc.dram_tensor("conv_b", [2, HID, 1], F32, kind="ExternalInput")
    headWT = nc.dram_tensor("headWT", [HID, 2 * LAT], BF16, kind="ExternalInput")
    head_b = nc.dram_tensor("head_b", [2 * LAT, 1], F32, kind="ExternalInput")
    iota = nc.dram_tensor("iota", [P, W, MAXBLK], BF16, kind="ExternalInput")
    edata = nc.dram_tensor("edata", [P, ecols], I16, kind="ExternalInput")
    muv_out = nc.dram_tensor("muvT", [2 * LAT, SHARD], F32, kind="ExternalOutput")

    ag_in = [nc.dram_tensor(f"ag_in{i}", [SHARD, HID], BF16) for i in range(2)]
    tables = [nc.dram_tensor(f"h_full{i}", [N_NODES, HID], BF16,
                             addr_space="Shared") for i in range(2)]

    chunk_ioff = []
    io = 0
    for m in metas:
        chunk_ioff.append(io)
        io += m["nblk"] * 10
    assert io == ecols

    with TileContext(nc) as tc:
        with (
            tc.tile_pool(name="const", bufs=1) as cp,
            tc.tile_pool(name="big", bufs=1) as bigp,
            tc.tile_pool(name="work", bufs=3) as wp,
            tc.tile_pool(name="msgp", bufs=2) as msgp,
            tc.tile_pool(name="swp", bufs=3) as swp,
            tc.tile_pool(name="ps_lin", bufs=3, space="PSUM") as ps_lin,
            tc.tile_pool(name="ps_tr", bufs=2, space="PSUM") as ps_tr,
            tc.tile_pool(name="ps_agg", bufs=3, space="PSUM") as ps_agg,
        ):
            # ---- constants ----
            w1t_sb = [cp.tile([P, HID], BF16, tag=f"w1_{k}", name=f"w1t_{k}") for k in range(4)]
            for k in range(4):
                nc.sync.dma_start(out=w1t_sb[k][:], in_=w1T[k * P:(k + 1) * P, :])
            w2t_sb = cp.tile([P, HID], BF16, tag="w2")
            nc.sync.dma_start(out=w2t_sb[:], in_=w2T[:, :])
            cw_sb = [[cp.tile([P, HID], BF16, tag=f"cw{l}{m}", name=f"cw_{l}_{m}") for m in range(2)]
                     for l in range(2)]
            for l in range(2):
                for m in range(2):
                    nc.scalar.dma_start(out=cw_sb[l][m][:], in_=conv_wT[l, m, :, :])
            b1_sb = cp.tile([P, 1], F32, tag="b1")
            nc.scalar.dma_start(out=b1_sb[:], in_=b1[:, :])
            b2_sb = cp.tile([P, 1], F32, tag="b2")
            nc.scalar.dma_start(out=b2_sb[:], in_=b2[:, :])
            cb_sb = [cp.tile([P, 1], F32, tag=f"cb{l}", name=f"cb_{l}") for l in range(2)]
            for l in range(2):
                nc.scalar.dma_start(out=cb_sb[l][:], in_=conv_b[l, :, :])
            hw_sb = cp.tile([P, 2 * LAT], BF16, tag="hw")
            nc.scalar.dma_start(out=hw_sb[:], in_=headWT[:, :])
            hb_sb = cp.tile([2 * LAT, 1], F32, tag="hb")
            nc.scalar.dma_start(out=hb_sb[:], in_=head_b[:, :])
            iota_sb = cp.tile([P, W, MAXBLK], BF16, tag="iota")
            nc.sync.dma_start(out=iota_sb[:], in_=iota[:, :, :])
            ident = cp.tile([P, P], F32, tag="ident")
            make_identity(nc, ident[:])
            identb = cp.tile([P, P], BF16, tag="identb")
            nc.vector.tensor_copy(out=identb[:], in_=ident[:])

            # all edge metadata (shared by both conv layers); loaded after the
            # MLP's x loads are queued
            edall = bigp.tile([P, ecols], I16, tag="edall")

            hA = bigp.tile([P, SHARD], BF16, tag="hA")   # h2T, then h4T
            hB = bigp.tile([P, SHARD], BF16, tag="hB")   # h3T
            aggT = bigp.tile([P, SHARD], BF16, tag="aggT")
            stage = bigp.tile([P, NB_STAGE * P], BF16, tag="stage")

            def emit_publish(hT_tile, t_idx, col, nt, eng=None):
                # transpose 128-col blocks into natural layout in `stage`,
                # then one batched DMA per tile into ag_in
                eng = eng or nc.sync
                b0 = col // P
                nfull = nt // P
                nblks = (nt + P - 1) // P
                for i in range(nblks):
                    w_ = min(P, nt - i * P)
                    tr_ps = ps_tr.tile([P, P], BF16, space="PSUM", tag="tr",
                                       name="trp")
                    nc.tensor.transpose(out=tr_ps[:w_, :],
                                        in_=hT_tile[:, col + i * P:col + i * P + w_],
                                        identity=identb[:])
                    nc.vector.tensor_copy(
                        out=stage[:w_, (b0 + i) * P:(b0 + i + 1) * P],
                        in_=tr_ps[:w_, :])
                if nfull:
                    eng.dma_start(
                        out=ag_in[t_idx][col:col + nfull * P, :].rearrange(
                            "(b p) f -> p b f", p=P),
                        in_=stage[:, col:col + nfull * P].rearrange(
                            "p (b f) -> p b f", f=P))
                if nt % P:
                    r0 = col + nfull * P
                    rb = b0 + nfull
                    eng.dma_start(
                        out=ag_in[t_idx][r0:r0 + nt % P, :],
                        in_=stage[:nt % P, rb * P:(rb + 1) * P])

            def emit_allgather(t_idx):
                nc.gpsimd.collective_compute(
                    "AllGather", mybir.AluOpType.bypass,
                    replica_groups=[list(range(N_CORES))],
                    ins=[ag_in[t_idx][:, :]],
                    outs=[tables[t_idx][:, :]],
                )

            def build_sw(ci):
                meta = metas[ci]
                nblk = meta["nblk"]
                io = chunk_ioff[ci]
                seg_t = edall[:, io + nblk * 8:io + nblk * 9].bitcast(BF16)
                w_t = edall[:, io + nblk * 9:io + nblk * 10].bitcast(BF16)
                s_w = swp.tile([P, W, MAXBLK], BF16, tag="sw")
                nc.vector.tensor_tensor(
                    out=s_w[:, :, :nblk],
                    in0=seg_t.unsqueeze(1).to_broadcast([P, W, nblk]),
                    in1=iota_sb[:, :, :nblk],
                    op=mybir.AluOpType.is_equal)
                nc.vector.tensor_tensor(
                    out=s_w[:, :, :nblk], in0=s_w[:, :, :nblk],
                    in1=w_t.unsqueeze(1).to_broadcast([P, W, nblk]),
                    op=mybir.AluOpType.mult)
                return s_w

            # ---- MLP (bf16 matmuls, f32 psum) ----
            xfp_cm = tc.tile_pool(name="xf", bufs=1)
            xfp = xfp_cm.__enter__()
            xfull = [xfp.tile([P, SHARD], BF16, tag=f"xf{k}", name=f"xf_{k}")
                     for k in range(4)]
            # column-chunked loads so the first MLP tile starts early, split
            # across the two HWDGE queues (SP and Activation)
            CCH = [(0, 1024), (1024, 2048), (2048, 3072), (3072, 4608),
                   (4608, SHARD)]
            for (c0, c1) in CCH:
                for k in range(4):
                    eng = nc.sync if k % 2 == 0 else nc.scalar
                    eng.dma_start(out=xfull[k][:, c0:c1],
                                  in_=xT[k * P:(k + 1) * P, c0:c1])
            col = 0
            for nt in N_TILES:
                h1_ps = ps_lin.tile([P, 512], F32, space="PSUM", tag="lin")
                for k in range(4):
                    nc.tensor.matmul(out=h1_ps[:, :nt], lhsT=w1t_sb[k][:],
                                     rhs=xfull[k][:, col:col + nt],
                                     start=(k == 0), stop=(k == 3))
                h1_sb = wp.tile([P, 512], BF16, tag="h1")
                # relu(psum + b1) as one fused DVE op, freeing the Activation
                # engine's queue for the x-load DMAs
                nc.vector.tensor_scalar(
                    out=h1_sb[:, :nt], in0=h1_ps[:, :nt], scalar1=b1_sb[:],
                    scalar2=0.0, op0=mybir.AluOpType.add,
                    op1=mybir.AluOpType.max)
                h2_ps = ps_lin.tile([P, 512], F32, space="PSUM", tag="lin")
                nc.tensor.matmul(out=h2_ps[:, :nt], lhsT=w2t_sb[:],
                                 rhs=h1_sb[:, :nt], start=True, stop=True)
                # h2 relu on DVE too: the Activation queue is occupied by the
                # x-load DMAs, and every publish (transpose/copy/DMA) cascades
                # behind this eviction
                nc.vector.tensor_scalar(
                    out=hA[:, col:col + nt], in0=h2_ps[:, :nt],
                    scalar1=b2_sb[:], scalar2=0.0,
                    op0=mybir.AluOpType.add, op1=mybir.AluOpType.max)
                emit_publish(hA, 0, col, nt)
                col += nt

            def conv_layer(layer, hT_in, hT_out, table, pub_idx=None,
                           heads=False, sw_cache=None):
                tiles_done = 0
                groups_done = 0

                def emit_linear(t):
                    col = 512 * t
                    nt = N_TILES[t]
                    ps = ps_lin.tile([P, 512], F32, space="PSUM", tag="lin")
                    nc.tensor.matmul(out=ps[:, :nt], lhsT=cw_sb[layer][0][:],
                                     rhs=aggT[:, col:col + nt],
                                     start=True, stop=False)
                    nc.tensor.matmul(out=ps[:, :nt], lhsT=cw_sb[layer][1][:],
                                     rhs=hT_in[:, col:col + nt],
                                     start=False, stop=True)
                    nc.scalar.activation(out=hT_out[:, col:col + nt],
                                         in_=ps[:, :nt],
                                         func=mybir.ActivationFunctionType.Relu,
                                         bias=cb_sb[layer][:])
                    if pub_idx is not None:
                        emit_publish(hT_out, pub_idx, col, nt)
                    if heads:
                        psh = ps_lin.tile([2 * LAT, 512], F32, space="PSUM",
                                          tag="lin", name="headps")
                        nc.tensor.matmul(out=psh[:, :nt], lhsT=hw_sb[:],
                                         rhs=hT_out[:, col:col + nt],
                                         start=True, stop=True)
                        muv = wp.tile([2 * LAT, 512], F32, tag="muv",
                                      name="muv")
                        nc.vector.tensor_tensor(
                            out=muv[:, :nt], in0=psh[:, :nt],
                            in1=hb_sb[:].to_broadcast([2 * LAT, nt]),
                            op=mybir.AluOpType.add)
                        nc.sync.dma_start(out=muv_out[:, col:col + nt],
                                          in_=muv[:, :nt])

                for ci, meta in enumerate(metas):
                    nblk, nblk_lo = meta["nblk"], meta["nblk_lo"]
                    io = chunk_ioff[ci]
                    idx_t = edall[:, io:io + nblk * 8]

                    msgL = msgp.tile([P, HCAP, HID], BF16, tag="msgL")
                    msgH = msgp.tile([P, HCAP, HID], BF16, tag="msgH")
                    if nblk_lo:
                        nc.gpsimd.dma_gather(
                            out_ap=msgL[:, :nblk_lo, :], in_ap=table[:HALF, :],
                            idxs_ap=idx_t[:, :nblk_lo * 8],
                            num_idxs=nblk_lo * P, num_idxs_reg=nblk_lo * P,
                            elem_size=HID, single_packet=False,
                            queue_num=0)
                    if nblk - nblk_lo:
                        nh = nblk - nblk_lo
                        nc.gpsimd.dma_gather(
                            out_ap=msgH[:, :nh, :], in_ap=table[HALF:, :],
                            idxs_ap=idx_t[:, nblk_lo * 8:nblk * 8],
                            num_idxs=nh * P, num_idxs_reg=nh * P,
                            elem_size=HID, single_packet=False,
                            queue_num=1)

                    if sw_cache is not None and ci in sw_cache:
                        s_w = sw_cache.pop(ci)
                    else:
                        s_w = build_sw(ci)

                    # one psum + one eviction per group: a group's lo and hi
                    # runs accumulate into the same tile
                    by_group = {}
                    for (g, h, b0, b1_) in meta["runs"]:
                        by_group.setdefault(g, []).append((h, b0, b1_))
                    for g in meta["groups"]:
                        ps = ps_agg.tile([P, W], F32, space="PSUM", tag="agg")
                        blocks = [(h, b) for (h, b0, b1_) in by_group[g]
                                  for b in range(b0, b1_)]
                        for i, (h, b) in enumerate(blocks):
                            mt = msgL[:, b, :] if h == 0 else \
                                 msgH[:, b - nblk_lo, :]
                            nc.tensor.matmul(out=ps[:], lhsT=mt,
                                             rhs=s_w[:, :, b],
                                             start=(i == 0),
                                             stop=(i == len(blocks) - 1))
                        gw = min(W, SHARD - g * W)
                        nc.scalar.activation(
                            out=aggT[:, g * W:g * W + gw], in_=ps[:, :gw],
                            func=mybir.ActivationFunctionType.Copy)
                    groups_done += len(meta["groups"])

                    while (tiles_done < len(N_TILES)
                           and groups_done * W >= tiles_done * 512
                           + N_TILES[tiles_done]):
                        emit_linear(tiles_done)
                        tiles_done += 1
                while tiles_done < len(N_TILES):
                    emit_linear(tiles_done)
                    tiles_done += 1

            xfp_cm.__exit__(None, None, None)
            nc.scalar.dma_start(out=edall[:], in_=edata[:, :])

            # prebuild the first S_w tiles while the AllGather is in flight
            sw_cache = {ci: build_sw(ci) for ci in range(min(3, len(metas)))}
            emit_allgather(0)
            conv_layer(0, hA, hB, tables[0], pub_idx=1, sw_cache=sw_cache)
            sw_cache = {ci: build_sw(ci) for ci in range(min(3, len(metas)))}
            emit_allgather(1)
            conv_layer(1, hB, hA, tables[1], heads=True, sw_cache=sw_cache)

    nc.finalize()
    return nc


# -------------------------------------------------------------------- driver --

def _get_compiled(x, edge_index, edge_attr, weights):
    src = np.asarray(edge_index[0]).astype(np.int64)
    dst = np.asarray(edge_index[1]).astype(np.int64)
    wgt = np.asarray(edge_attr, dtype=np.float32)
    x = np.asarray(x, dtype=np.float32)

    per_core_edges = []
    for c in range(N_CORES):
        sel = (dst >= c * SHARD) & (dst < (c + 1) * SHARD)
        s, d, wv = src[sel], dst[sel] - c * SHARD, wgt[sel]
        order = np.argsort(d, kind="stable")
        per_core_edges.append((s[order], d[order], wv[order]))

    metas, edata = _unified_structure(per_core_edges)
    ecols = sum(m["nblk"] * 10 for m in metas)

    nc = _build(metas, ecols)

    (W1, b1, W2, b2, g1_rel_W, g1_rel_b, g1_root_W,
     g2_rel_W, g2_rel_b, g2_root_W, mu_W, mu_b, lv_W, lv_b) = [
        np.asarray(w, dtype=np.float32) for w in weights]

    conv_wT = np.stack([
        np.stack([g1_rel_W.T, g1_root_W.T]),
        np.stack([g2_rel_W.T, g2_root_W.T]),
    ]).astype(NP_BF16).copy()
    conv_b = np.stack([g1_rel_b[:, None], g2_rel_b[:, None]]).copy()
    headWT = np.ascontiguousarray(
        np.concatenate([mu_W, lv_W], axis=0).T.astype(NP_BF16))
    head_b = np.concatenate([mu_b, lv_b])[:, None].copy()
    iota = np.ascontiguousarray(np.broadcast_to(
        np.arange(W, dtype=np.float32)[None, :, None],
        (P, W, MAXBLK)).astype(NP_BF16))

    common = dict(
        w1T=np.ascontiguousarray(W1.T.astype(NP_BF16)), b1=b1[:, None].copy(),
        w2T=np.ascontiguousarray(W2.T.astype(NP_BF16)), b2=b2[:, None].copy(),
        conv_wT=conv_wT, conv_b=conv_b, headWT=headWT, head_b=head_b,
        iota=iota,
    )
    in_maps = []
    for c in range(N_CORES):
        m = dict(common)
        m["xT"] = np.ascontiguousarray(x[c * SHARD:(c + 1) * SHARD, :].T.astype(NP_BF16))
        m["edata"] = edata[c]
        in_maps.append(m)
    return nc, in_maps


def kernel(x, edge_index, edge_attr,
           W1, b1, W2, b2,
           g1_rel_W, g1_rel_b, g1_root_W,
           g2_rel_W, g2_rel_b, g2_root_W,
           mu_W, mu_b, lv_W, lv_b):
    weights = (W1, b1, W2, b2, g1_rel_W, g1_rel_b, g1_root_W,
               g2_rel_W, g2_rel_b, g2_root_W, mu_W, mu_b, lv_W, lv_b)
    nc, in_maps = _get_compiled(x, edge_index, edge_attr, weights)
    res = bass_utils.run_bass_kernel_spmd(nc, in_maps,
                                          core_ids=list(range(N_CORES)))
    muvT = np.concatenate([res.results[c]["muvT"] for c in range(N_CORES)],
                          axis=1)
    return (np.ascontiguousarray(muvT[:LAT, :].T),
            np.ascontiguousarray(muvT[LAT:, :].T))
